# revision 31
# baseline (speedup 1.0000x reference)
"""CPPN MLP (12 -> 32 -> 32 -> 32 -> 3, per-node activations) on 8 TRN2 cores.

Data-parallel over the pixel axis. Each core processes P_CORE pixels laid out
feature-major as 4 pixel-groups on SBUF partitions:
  rhs partition (12*g + i) holds feature i of pixel-group g  (layer-1 input)
  hidden state partition layout per layer: 4 groups x 32 nodes, nodes sorted
  [gauss | sin | tanh-class] across groups.

All matmul data (x, weights, hidden state h) is fp16: full-rate PE matmuls,
half the DMA bytes, 10-bit mantissa (~5e-4 relative) which the 2e-2 harness
gate easily absorbs.  PSUM accumulation stays fp32.

The tanh-class (tanh/sigmoid/identity) is handled by ONE Tanh pass over all
128 partitions with per-partition scale/bias operands plus host-side
algebraic folds into the next layer's weights:
  sigmoid(z) = 0.5*tanh(z/2) + 0.5          (stored tanh(z/2); affine folded)
  identity(z) = tanh(eps*z)/eps             (stored tanh(eps*z); 1/eps folded)

Sin and gauss rows are only a fraction of the partitions but a sub-range
activation op costs the same as a full-height one (cost ~ free-dim length).
So the main Tanh pass writes those rows as the identity-eps encoding
tanh(eps*(u+b)) ~= eps*(u+b) (eps = 2^-9).  Each hidden level h lives in ONE
persistent ring tile [128, RING*CHUNK], so a whole pack-group of chunks is a
contiguous column slice: the class rows of pf consecutive chunks are moved
into a densely packed tile [4*n*pf, CHUNK] with a SINGLE gather DMA (the
partition/free reshape falls out of DMA flatten-order pairing), the per-class
op chains run once per packed tile (amortized pf-fold), and one scatter DMA
writes the results back over the eps-junk rows of all pf chunks:
  gauss(z) = exp(-(z+b)^2/2):  DVE squares the encoding (y = enc^2, fp32);
    t = Tanh((0.25/eps^2)*y) = tanh(((u+b)/2)^2);  gauss = 2/(1+t) - 1 via
    DVE add + reciprocal_approx_fast + one affine tensor_scalar (also the
    fp32->fp16 convert).  Square runs on DVE, not ScalarE: ScalarE is the
    bottleneck engine (3 main passes + sin + gauss tanh + out tanh).
  sin(z+b):  ADD_RANGE_WRAP wraps the encoding into [-eps*pi, eps*pi] (the
    wrap is linear so it works in eps-space; one period suffices since
    |z+b| < 3*pi), then Sin decodes with scale 1/eps.
Pack factors are divisors of RING so a group never wraps the ring.  Junk rows
above the packed region flow through every op harmlessly.  The output stores
write the full [128, 512] quadrant-packed tile per chunk-pair (junk rows
included) so each is ONE descriptor; the host unpacks.  DMA issue is spread
across sequencers (gauss gathers/scatters + output stores on SP's DGE; x
loads and sin gathers/scatters on GpSimd's; none on Activation) so no
sequencer's DIRECT2D issue cost (~0.7us each) starves ScalarE.
"""

import os
import sys

import numpy as np

_REPO = "/root/.axon_site/_ro/trn_rl_repo"
if _REPO not in sys.path and not os.path.isdir("/opt/trn_rl_repo"):
    sys.path.insert(0, _REPO)

import concourse.bacc as bacc
import concourse.bass as bass  # noqa: F401
import concourse.tile as tile
from concourse import mybir
from concourse.bass_utils import run_bass_kernel_spmd

# Pin the activation-function table to the single set containing every
# function this kernel uses ({Tanh, Sin}).  Without this, bacc's greedy
# per-instruction set selection can alternate between sets and emit an
# ACT_TABLE_LOAD (~2.7us) per chunk.
_orig_get_tables = bacc.get_activation_tables


def _pinned_tables(arch):
    t = _orig_get_tables(arch)
    if "silu_and_others" in t:
        # act_func_set_id is the POSITION in act_info.json's set list, so
        # keep every entry (order intact) and just empty the others.
        return {name: (funcs if name == "silu_and_others" else set())
                for name, funcs in t.items()}
    return t


bacc.get_activation_tables = _pinned_tables

F32 = mybir.dt.float32
F16 = mybir.dt.float16

P_TOTAL = 1024 * 1024
N_IN, H, N_OUT = 12, 32, 3
N_CORES = 8
P_CORE = P_TOTAL // N_CORES  # 131072
G = 4                        # pixel groups packed on partitions
PG = P_CORE // G             # 32768 pixels per group per core
CHUNK = 1024                 # pixels per group per chunk (2 PSUM banks)
MM_N = 512                   # matmul moving free dim (one PSUM bank)
RING = 12                    # h ring depth (chunks) per hidden level
ID_EPS = np.float32(2.0 ** -9)      # identity-via-tanh input scale
TWO_PI = float(2.0 * np.pi)
PI = float(np.pi)


def _pack_factor(n):
    """Chunks packed per class tile for a class with n nodes (4n rows).
    Must divide RING so groups never wrap the h ring."""
    if n == 0:
        return 0
    cap = 128 // (4 * n)
    for pf in (6, 4, 3, 2, 1):
        if pf <= cap:
            return pf
    return 1


# class codes: 0 = gauss, 1 = sin, 2 = tanh-class (tanh/sigmoid/identity)
def _cls_of_act(a):
    return {4: 0, 3: 1}.get(int(a), 2)


def _sorted_layout(act):
    """Order the H nodes by [gauss | sin | rest]; return (perm, n_gauss, n_sin).
    perm[j] = original node index placed at sorted slot j."""
    cls = np.array([_cls_of_act(a) for a in act])
    perm = np.argsort(cls, kind="stable")
    return perm, int((cls == 0).sum()), int((cls == 1).sum())


class _Plan:
    """Host-side folded weights + per-layer layouts. All float64 math."""

    def __init__(self, bias_in, W1, b1, act1, W2, b2, act2, W3, b3, act3,
                 Wout, bout):
        layers = [(W1, b1, act1), (W2, b2, act2), (W3, b3, act3)]
        self.perms, self.ngauss, self.nsin = [], [], []
        self.lhsT = []          # device stationary matrices (np.float32)
        self.cols = []          # per-layer dict of [128] operand columns
        # incoming per-node output transform: h_true = alpha*stored + beta
        in_alpha = np.ones(N_IN, dtype=np.float64)
        in_beta = np.asarray(bias_in, dtype=np.float64)  # h0 = x + bias_in
        in_dim = N_IN
        in_layout = None  # for L1 the input layout is the fixed feature order

        for li, (W, b, act) in enumerate(layers):
            W = np.asarray(W, dtype=np.float64)
            b = np.asarray(b, dtype=np.float64)
            act = np.asarray(act)
            perm, ng, ns = _sorted_layout(act)
            self.perms.append(perm)
            self.ngauss.append(ng)
            self.nsin.append(ns)

            # effective weights / bias absorbing incoming transforms
            W_eff = W * in_alpha[:, None]                  # [in_dim, H]
            b_eff = b + in_beta @ W                        # [H]

            # device stationary: block diagonal over groups with node sort
            K = G * in_dim
            lt = np.zeros((K, 128), dtype=np.float64)
            for g in range(G):
                for j in range(H):
                    node = perm[j]
                    m = self._row(li, g, j)
                    if li == 0:
                        rows = np.arange(in_dim) + in_dim * g
                        lt[rows, m] = W_eff[:, node]
                    else:
                        for k_in in range(in_dim):
                            kpart = in_layout[g][k_in]
                            lt[kpart, m] = W_eff[k_in, node]
            self.lhsT.append(lt.astype(np.float32))

            # operand columns.  Main tanh pass: per-partition scale/bias.
            tanh_scale = np.zeros(128, dtype=np.float64)
            tanh_bias = np.zeros(128, dtype=np.float64)
            out_alpha = np.ones(H, dtype=np.float64)
            out_beta = np.zeros(H, dtype=np.float64)
            for j in range(H):
                node = perm[j]
                a = int(act[node])
                be = b_eff[node]
                for g in range(G):
                    m = self._row(li, g, j)
                    if a == 1:        # tanh
                        tanh_scale[m] = 1.0
                        tanh_bias[m] = be
                    elif a == 2:      # sigmoid -> tanh(u/2)
                        tanh_scale[m] = 0.5
                        tanh_bias[m] = 0.5 * be
                    else:
                        # identity nodes AND the sin/gauss rows: the main
                        # tanh pass writes the identity-eps encoding
                        # tanh(eps*(u+b)) ~= eps*(u+b), which for sin/gauss
                        # is the value the packed chains gather from h
                        # (DMA cannot read PSUM).
                        tanh_scale[m] = float(ID_EPS)
                        tanh_bias[m] = float(ID_EPS) * be
                if a == 1:
                    out_alpha[node], out_beta[node] = 1.0, 0.0
                elif a == 2:
                    out_alpha[node], out_beta[node] = 0.5, 0.5
                elif a == 0:
                    out_alpha[node], out_beta[node] = 1.0 / float(ID_EPS), 0.0
                else:                 # sin / gauss: stored value is exact
                    out_alpha[node], out_beta[node] = 1.0, 0.0
            self.cols.append({
                "tanh_scale": tanh_scale, "tanh_bias": tanh_bias,
            })

            # next layer's incoming transform, in SORTED node order per device
            # partition -> but folds are per node; store per-node arrays and
            # the partition layout for the next lhsT build.
            in_alpha = out_alpha
            in_beta = out_beta
            in_dim = H
            # partition index of (g, sorted-slot j) for this layer's output
            in_layout = [[self._row(li, g, j) for j in range(H)]
                         for g in range(G)]
            # reorder alpha/beta to sorted-slot order for the next W_eff
            in_alpha = out_alpha[perm]
            in_beta = out_beta[perm]
            # next layer's W rows must be permuted accordingly
            if li < 2:
                layers[li + 1] = (np.asarray(layers[li + 1][0])[perm, :],
                                  layers[li + 1][1], layers[li + 1][2])
            else:
                self._wout_perm = perm

        # output layer
        Wo = np.asarray(Wout, dtype=np.float64)[self._wout_perm, :]
        bo = np.asarray(bout, dtype=np.float64)
        Wo_eff = Wo * in_alpha[:, None]
        bo_eff = bo + in_beta @ Wo
        lt = np.zeros((128, 32), dtype=np.float64)
        for g in range(G):
            for j in range(H):
                kpart = in_layout[g][j]
                for o in range(N_OUT):
                    lt[kpart, 3 * g + o] = Wo_eff[j, o]
        self.lhsT_out = lt.astype(np.float32)
        out_bias = np.zeros(128, dtype=np.float64)
        for q in range(4):
            for g in range(G):
                for o in range(N_OUT):
                    out_bias[32 * q + 3 * g + o] = bo_eff[o]
        self.out_bias = out_bias

        # pack all operand columns into one [128, 32] block
        colblk = np.zeros((128, 32), dtype=np.float64)
        for li in range(3):
            c = self.cols[li]
            colblk[:, 8 * li + 0] = c["tanh_scale"]
            colblk[:, 8 * li + 1] = c["tanh_bias"]
        colblk[:, 24] = self.out_bias
        self.colblk = colblk.astype(np.float32)

    @staticmethod
    def _row(li, g, j):
        """Device partition of sorted-slot j, group g (layer output layout).
        Rows are class-sorted ACROSS groups: slot j occupies partitions
        4*j + g."""
        return 4 * j + g


def _build_program(ngauss, nsin, p_core=P_CORE, chunk=CHUNK,
                   use_fp32r=True):
    """Build the bass module. Program structure depends only on the per-layer
    (n_gauss, n_sin) counts, not on weight values."""
    pg = p_core // G
    nchunk = pg // chunk
    nhalf = chunk // MM_N
    assert chunk % MM_N == 0 and pg % chunk == 0

    pfg = [_pack_factor(n) for n in ngauss]   # gauss pack factor per layer
    pfs = [_pack_factor(n) for n in nsin]     # sin pack factor per layer
    # emission skew between layers: covers each layer's largest pack-group
    # latency (a chunk's h completes only when its packed group completes;
    # skew is a priority hint, dataflow is dependency-enforced).  +1 for the
    # deferred chain phase B, +2 slack.
    s1 = max(pfg[0], pfs[0]) + 2
    s2 = s1 + max(pfg[1], pfs[1]) + 2
    s3 = s2 + max(pfg[2], pfs[2]) + 2
    skew = [0, s1, s2, s3]
    total_skew = skew[3] + 2

    nc = bacc.Bacc("TRN2", target_bir_lowering=False, debug=False,
                   num_devices=N_CORES)
    xT = nc.dram_tensor("xT", [G * N_IN, pg], F16, kind="ExternalInput").ap()
    wst = nc.dram_tensor("wst", [128, 416], F16, kind="ExternalInput").ap()
    cst = nc.dram_tensor("cst", [128, 64], F32, kind="ExternalInput").ap()
    yT = nc.dram_tensor("yT", [128, pg // 4], F32, kind="ExternalOutput").ap()

    with tile.TileContext(nc) as tc:
        cpool = tc.alloc_tile_pool(name="consts", bufs=1)
        wst_t = cpool.tile([128, 416], F16, tag="wst")
        cc_t = cpool.tile([128, 32], F32, tag="cc")
        nc.sync.dma_start(out=wst_t[:], in_=wst[:, 0:416])
        nc.sync.dma_start(out=cc_t[:], in_=cst[:, 0:32])
        w1_t = wst_t[:, 0:128]
        w2_t = wst_t[:, 128:256]
        w3_t = wst_t[:, 256:384]
        wo_t = wst_t[:, 384:416]
        col_t = cc_t[:, 0:32]

        ring = min(RING, nchunk)
        # persistent h ring tiles, one per hidden level (subtile-dep tracked)
        h1_t = cpool.tile([128, ring * chunk], F16, tag="h1")
        h2_t = cpool.tile([128, ring * chunk], F16, tag="h2")
        h3_t = cpool.tile([128, ring * chunk], F16, tag="h3")
        h_ring = [None, h1_t, h2_t, h3_t]

        # one SBUF work pool + one PSUM pool (per-tag bufs); fewer pools =
        # fewer release-barrier ceremonies in the teardown
        wpool = tc.alloc_tile_pool(name="work", bufs=2)
        xpool = gpool = spool = scpool = rpool = opool = wpool
        ppool = tc.alloc_tile_pool(name="psum", bufs=3, space="PSUM")
        oppool = ppool

        w_tiles = [w1_t, w2_t, w3_t]
        x_live = {}     # chunk -> x tile
        pso_live = {}   # chunk-pair -> psum_o tile
        def _new_cst():
            return {"pend": [], "subs": [], "row": 0, "npf": 0, "done": 0,
                    "tile": None}

        gst = {li: _new_cst() for li in range(3)}
        sst = {li: _new_cst() for li in range(3)}

        def hsl(c, n=1):
            """Column slice of n consecutive chunks starting at c (no wrap:
            pack factors divide the ring depth)."""
            r = c % ring
            assert r + n <= ring
            return slice(r * chunk, (r + n) * chunk)

        def emit_load(c):
            x_t = xpool.tile([G * N_IN, chunk], F16, tag="x", bufs=4)
            nc.gpsimd.dma_start(
                out=x_t[:], in_=xT[:, c * chunk:(c + 1) * chunk])
            x_live[c] = x_t

        def emit_main(c, li):
            """Main matmuls + full-height tanh pass."""
            if li == 0:
                h_prev = x_live.pop(c)
            else:
                h_prev = h_ring[li][:, hsl(c)]
            kdim = G * N_IN if li == 0 else 128
            ps = ppool.tile([128, chunk], F32, tag="pre")
            wt = w_tiles[li]
            for hh in range(nhalf):
                sl = slice(hh * MM_N, (hh + 1) * MM_N)
                nc.tensor.matmul(
                    ps[:, sl],
                    wt[0:kdim, :],
                    h_prev[0:kdim, sl],
                    start=True, stop=True,
                )
            cb = 8 * li
            # tanh-class pass over all 128 rows (junk eps-encode on the
            # gauss/sin rows, overwritten by the packed-chain scatters)
            nc.scalar.activation(
                h_ring[li + 1][:, hsl(c)], ps[:],
                mybir.ActivationFunctionType.Tanh,
                bias=col_t[:, cb + 1:cb + 2],
                scale=col_t[:, cb + 0:cb + 1],
            )
            # incremental sub-group gathers into the packed class tiles:
            # each group of npf chunks is gathered in two sub-DMAs (rows
            # [off : off + 4n*L] <- [4n, L*C], the reshape falls out of the
            # DMA flatten-order pairing), so the packed data is ready right
            # after the group's last main pass and every AP is a contiguous
            # row range (soundly dependency-tracked).
            ng, ns = ngauss[li], nsin[li]
            if ng > 0:
                self_gather(gst[li], li, c, pfg[li], nc.sync, gpool,
                            f"gz{li}", 0, 4 * ng)
            if ns > 0:
                self_gather(sst[li], li, c, pfs[li], nc.gpsimd, spool,
                            f"sz{li}", 4 * ng, 4 * ns)

        def self_gather(st, li, c, pf, eng, pool, tag, rbase, rows):
            """Append chunk c to the class group; gather a sub when half the
            group (or the remainder) has accumulated."""
            if st["tile"] is None:
                st["tile"] = pool.tile([128, chunk], F16, tag=tag, name=tag)
                st["npf"] = min(pf, nchunk - c)
                st["row"] = 0
                st["done"] = 0
                st["subs"] = []
                st["pend"] = []
            st["pend"].append(c)
            # sub lengths: ceil(npf/2) then the rest
            first = (st["npf"] + 1) // 2
            want = first if st["done"] == 0 else st["npf"] - first
            if len(st["pend"]) == want:
                L = want
                c_start = st["pend"][0]
                off = st["row"]
                eng.dma_start(
                    out=st["tile"][off:off + rows * L, :],
                    in_=h_ring[li + 1][rbase:rbase + rows, hsl(c_start, L)])
                st["subs"].append((off, c_start, L))
                st["row"] = off + rows * L
                st["done"] += L
                st["pend"] = []

        def scatter_subs(eng, res, li, rbase, rows, subs):
            for off, c_start, L in subs:
                eng.dma_start(
                    out=h_ring[li + 1][rbase:rbase + rows, hsl(c_start, L)],
                    in_=res[off:off + rows * L, :])

        def gauss_head(li):
            """Phase A: DVE square of the packed encodings (gathers already
            landed incrementally).  Returns phase-B state."""
            st = gst[li]
            R = st["row"]
            gz = st["tile"]
            subs = st["subs"]
            st["tile"] = None
            # y = enc^2 on DVE (fp32; the (0.25/eps^2) decode folds into the
            # Tanh scale) - keeps Square off the bottleneck ScalarE
            ysq = scpool.tile([128, chunk], F32, tag="gy", bufs=6)
            nc.vector.tensor_tensor(ysq[0:R, :], gz[0:R, :], gz[0:R, :],
                                    mybir.AluOpType.mult)
            return (li, ysq, subs, R)

        def gauss_act(st):
            """Phase B1: t = tanh(((u+b)/2)^2) on ScalarE."""
            li, ysq, subs, R = st
            t_t = scpool.tile([128, chunk], F32, tag="gt", bufs=3)
            nc.scalar.activation(
                t_t[0:R, :], ysq[0:R, :], mybir.ActivationFunctionType.Tanh,
                scale=float(0.25 / (ID_EPS * ID_EPS)),
            )
            return (li, t_t, subs, R)

        def gauss_tail(st):
            """Phase B2: den = 1 + t ; r = 1/den ; out = 2r - 1 =
            exp(-(z+b)^2/2); scatter the subs back."""
            li, t_t, subs, R = st
            # den = 1 + t, in place (DVE element-wise streaming)
            nc.vector.tensor_scalar(
                t_t[0:R, :], t_t[0:R, :], 1.0, None, mybir.AluOpType.add)
            rin_t = scpool.tile([128, chunk], F32, tag="gr", bufs=3)
            nc.vector.reciprocal_approx_fast(rin_t[0:R, :], t_t[0:R, :])
            g_r = rpool.tile([128, chunk], F16, tag="go", bufs=3)
            nc.vector.tensor_scalar(
                g_r[0:R, :], rin_t[0:R, :], 2.0, -1.0,
                mybir.AluOpType.mult, mybir.AluOpType.add)
            scatter_subs(nc.sync, g_r, li, 0, 4 * ngauss[li], subs)

        def sin_head(li):
            """Phase A: DVE range-wrap of the packed encodings in eps-space."""
            st = sst[li]
            R = st["row"]
            sz = st["tile"]
            subs = st["subs"]
            st["tile"] = None
            m_t = scpool.tile([128, chunk], F32, tag="sm", bufs=6)
            nc.vector.add_range_wrap(
                m_t[0:R, :], sz[0:R, :],
                0.0, float(ID_EPS) * PI, float(ID_EPS) * TWO_PI)
            return (li, m_t, subs, R)

        def sin_act(st):
            """Phase B: Sin decodes with scale 1/eps; scatter the subs."""
            li, m_t, subs, R = st
            s_r = rpool.tile([128, chunk], F16, tag="so", bufs=3)
            nc.scalar.activation(
                s_r[0:R, :], m_t[0:R, :], mybir.ActivationFunctionType.Sin,
                scale=float(1.0 / ID_EPS))
            scatter_subs(nc.gpsimd, s_r, li, 4 * ngauss[li], 4 * nsin[li],
                         subs)

        def collect_flushes(c, li, heads):
            """After emit_main(c, li): start phase A for completed groups."""
            if ngauss[li] > 0 and gst[li]["tile"] is not None \
                    and gst[li]["done"] == gst[li]["npf"]:
                heads.append(("g", gauss_head(li)))
            if nsin[li] > 0 and sst[li]["tile"] is not None \
                    and sst[li]["done"] == sst[li]["npf"]:
                heads.append(("s", sin_head(li)))

        def emit_out(c):
            # output layer: quadrant-packed [12,512] matmuls
            h_prev = h_ring[3][:, hsl(c)]
            q0 = 2 * (c % 2)
            if q0 == 0:
                pso_live[c // 2] = oppool.tile([128, MM_N], F32, tag="preo",
                                               name="pso", bufs=2)
            pso = pso_live[c // 2]
            for hh in range(nhalf):
                q = q0 + hh
                nc.tensor.matmul(
                    pso[32 * q:32 * q + 32, :],
                    wo_t,
                    h_prev[:, hh * MM_N:(hh + 1) * MM_N],
                    start=True, stop=True,
                    tile_position=(0, 32 * q),
                )
            if q0 == 2:
                pso_live.pop(c // 2)
                osb = opool.tile([128, MM_N], F32, tag="osb")
                nc.scalar.activation(
                    osb[:], pso[:],
                    mybir.ActivationFunctionType.Tanh,
                    bias=col_t[:, 24:25],
                )
                # one full-height store per chunk-pair; host unpacks the
                # quadrant layout (junk rows included)
                k = c // 2
                nc.sync.dma_start(
                    out=yT[:, k * MM_N:(k + 1) * MM_N], in_=osb[:])

        # Software-pipelined emission with per-layer skew covering the packed
        # group latency.  Step order [L2, L3, L1, out] keeps the out-layer
        # matmuls (which wait on L3 chain scatters) from head-of-line
        # blocking L1's matmuls in the PE FIFO, and defers each chain's
        # ScalarE op (phase B) one full step behind its DVE head (phase A)
        # so it is ready when the ACT queue reaches it.
        assert nchunk % 2 == 0
        emit_load(0)
        pending = []   # phase-A states from the previous step
        for t in range(nchunk + total_skew):
            heads = []
            if skew[1] <= t and t - skew[1] < nchunk:
                emit_main(t - skew[1], 1)
                collect_flushes(t - skew[1], 1, heads)
            if skew[2] <= t and t - skew[2] < nchunk:
                emit_main(t - skew[2], 2)
                collect_flushes(t - skew[2], 2, heads)
            if t + 1 < nchunk:
                emit_load(t + 1)
            if t < nchunk:
                emit_main(t, 0)
                collect_flushes(t, 0, heads)
            if skew[3] <= t and t - skew[3] < nchunk:
                emit_out(t - skew[3])
            # phase B for last step's groups: ACT parts first, tails after
            bstates = []
            for kind, st in pending:
                bstates.append((kind, gauss_act(st) if kind == "g" else st))
            for kind, st in bstates:
                if kind == "g":
                    gauss_tail(st)
                else:
                    sin_act(st)
            pending = heads

        for p in (ppool, wpool, cpool):
            p.release()

    nc.compile()
    return nc


_PROGRAM_CACHE = {}


def _get_program(ngauss, nsin, p_core=P_CORE, chunk=CHUNK, use_fp32r=True):
    key = (tuple(ngauss), tuple(nsin), p_core, chunk, use_fp32r)
    if key not in _PROGRAM_CACHE:
        _PROGRAM_CACHE[key] = _build_program(ngauss, nsin, p_core, chunk,
                                             use_fp32r=use_fp32r)
    return _PROGRAM_CACHE[key]


def make_in_maps(inputs, plan, p_core=P_CORE, n_cores=N_CORES):
    """Shard + transpose the pixel data; replicate constants."""
    x = np.ascontiguousarray(np.asarray(inputs["inputs"], dtype=np.float32))
    pg = p_core // G
    in_maps = []
    for core in range(n_cores):
        xc = x[core * p_core:(core + 1) * p_core]          # [p_core, 12]
        xg = xc.reshape(G, pg, N_IN)                        # [G, pg, 12]
        xT = np.ascontiguousarray(xg.transpose(0, 2, 1)
                                  .reshape(G * N_IN, pg)
                                  .astype(np.float16))      # [48, pg]
        wst = np.zeros((128, 416), dtype=np.float16)
        wst[0:G * N_IN, 0:128] = plan.lhsT[0]
        wst[:, 128:256] = plan.lhsT[1]
        wst[:, 256:384] = plan.lhsT[2]
        wst[:, 384:416] = plan.lhsT_out
        cst = np.zeros((128, 64), dtype=np.float32)
        cst[:, 0:32] = plan.colblk
        in_maps.append({"xT": xT, "wst": wst, "cst": cst})
    return in_maps


def assemble_output(results, p_core=P_CORE, n_cores=N_CORES):
    pg = p_core // G
    nk = pg // (2 * CHUNK) if pg >= 2 * CHUNK else 1
    out = np.empty((p_core * n_cores, N_OUT), dtype=np.float32)
    for core in range(n_cores):
        yT = results[core]["yT"]                     # [128, pg/4]
        # quadrant layout: yT[32q + 3g + o, 512k + j] is (g, o) of slot
        # s = 2048k + 512q + j
        arr = yT.reshape(4, 32, nk, MM_N)            # [q, row, k, j]
        yc = (arr[:, 0:12]                           # [q, 3g+o, k, j]
              .transpose(1, 2, 0, 3)                 # [3g+o, k, q, j]
              .reshape(G, N_OUT, pg)                 # [g, o, slot]
              .transpose(0, 2, 1))                   # [g, slot, o]
        out[core * p_core:(core + 1) * p_core] = yc.reshape(p_core, N_OUT)
    return out


def make_plan(inputs):
    return _Plan(
        inputs["bias_in"], inputs["W1"], inputs["b1"], inputs["act1"],
        inputs["W2"], inputs["b2"], inputs["act2"],
        inputs["W3"], inputs["b3"], inputs["act3"],
        inputs["Wout"], inputs["bout"])


def run(inputs, trace=False, use_fp32r=True, **spmd_kwargs):
    plan = make_plan(inputs)
    nc = _get_program(plan.ngauss, plan.nsin, use_fp32r=use_fp32r)
    in_maps = make_in_maps(inputs, plan)
    res = run_bass_kernel_spmd(nc, in_maps, list(range(N_CORES)),
                               trace=trace, **spmd_kwargs)
    return assemble_output(res.results), res


def kernel(**inputs) -> np.ndarray:
    out, _ = run(inputs, trace=False)
    return out


# revision 32
# speedup vs baseline: 1.0196x; 1.0196x over previous
"""CPPN MLP (12 -> 32 -> 32 -> 32 -> 3, per-node activations) on 8 TRN2 cores.

Data-parallel over the pixel axis. Each core processes P_CORE pixels laid out
feature-major as 4 pixel-groups on SBUF partitions:
  rhs partition (12*g + i) holds feature i of pixel-group g  (layer-1 input)
  hidden state partition layout per layer: 4 groups x 32 nodes, nodes sorted
  [gauss | sin | tanh-class] across groups.

All matmul data (x, weights, hidden state h) is fp16: full-rate PE matmuls,
half the DMA bytes, 10-bit mantissa (~5e-4 relative) which the 2e-2 harness
gate easily absorbs.  PSUM accumulation stays fp32.

The tanh-class (tanh/sigmoid/identity) is handled by ONE Tanh pass over all
128 partitions with per-partition scale/bias operands plus host-side
algebraic folds into the next layer's weights:
  sigmoid(z) = 0.5*tanh(z/2) + 0.5          (stored tanh(z/2); affine folded)
  identity(z) = tanh(eps*z)/eps             (stored tanh(eps*z); 1/eps folded)

Sin and gauss rows are only a fraction of the partitions but a sub-range
activation op costs the same as a full-height one (cost ~ free-dim length).
So the main Tanh pass writes those rows as the identity-eps encoding
tanh(eps*(u+b)) ~= eps*(u+b) (eps = 2^-9).  Each hidden level h lives in ONE
persistent ring tile [128, RING*CHUNK], so a whole pack-group of chunks is a
contiguous column slice: the class rows of pf consecutive chunks are moved
into a densely packed tile [4*n*pf, CHUNK] with a SINGLE gather DMA (the
partition/free reshape falls out of DMA flatten-order pairing), the per-class
op chains run once per packed tile (amortized pf-fold), and one scatter DMA
writes the results back over the eps-junk rows of all pf chunks:
  gauss(z) = exp(-(z+b)^2/2):  DVE squares the encoding (y = enc^2, fp32);
    t = Tanh((0.25/eps^2)*y) = tanh(((u+b)/2)^2);  gauss = 2/(1+t) - 1 via
    DVE add + reciprocal_approx_fast + one affine tensor_scalar (also the
    fp32->fp16 convert).  Square runs on DVE, not ScalarE: ScalarE is the
    bottleneck engine (3 main passes + sin + gauss tanh + out tanh).
  sin(z+b):  ADD_RANGE_WRAP wraps the encoding into [-eps*pi, eps*pi] (the
    wrap is linear so it works in eps-space; one period suffices since
    |z+b| < 3*pi), then Sin decodes with scale 1/eps.
Pack factors are divisors of RING so a group never wraps the ring.  Junk rows
above the packed region flow through every op harmlessly.  The output stores
write the full [128, 512] quadrant-packed tile per chunk-pair (junk rows
included) so each is ONE descriptor; the host unpacks.  DMA issue is spread
across sequencers (gauss gathers/scatters + output stores on SP's DGE; x
loads and sin gathers/scatters on GpSimd's; none on Activation) so no
sequencer's DIRECT2D issue cost (~0.7us each) starves ScalarE.
"""

import os
import sys

import numpy as np

_REPO = "/root/.axon_site/_ro/trn_rl_repo"
if _REPO not in sys.path and not os.path.isdir("/opt/trn_rl_repo"):
    sys.path.insert(0, _REPO)

import concourse.bacc as bacc
import concourse.bass as bass  # noqa: F401
import concourse.tile as tile
from concourse import mybir
from concourse.bass_utils import run_bass_kernel_spmd

# Pin the activation-function table to the single set containing every
# function this kernel uses ({Tanh, Sin}).  Without this, bacc's greedy
# per-instruction set selection can alternate between sets and emit an
# ACT_TABLE_LOAD (~2.7us) per chunk.
_orig_get_tables = bacc.get_activation_tables


def _pinned_tables(arch):
    t = _orig_get_tables(arch)
    if "silu_and_others" in t:
        # act_func_set_id is the POSITION in act_info.json's set list, so
        # keep every entry (order intact) and just empty the others.
        return {name: (funcs if name == "silu_and_others" else set())
                for name, funcs in t.items()}
    return t


bacc.get_activation_tables = _pinned_tables

F32 = mybir.dt.float32
F16 = mybir.dt.float16

P_TOTAL = 1024 * 1024
N_IN, H, N_OUT = 12, 32, 3
N_CORES = 8
P_CORE = P_TOTAL // N_CORES  # 131072
G = 4                        # pixel groups packed on partitions
PG = P_CORE // G             # 32768 pixels per group per core
CHUNK = 1024                 # pixels per group per chunk (2 PSUM banks)
MM_N = 512                   # matmul moving free dim (one PSUM bank)
RING = 12                    # h ring depth (chunks) per hidden level
ID_EPS = np.float32(2.0 ** -9)      # identity-via-tanh input scale
TWO_PI = float(2.0 * np.pi)
PI = float(np.pi)


def _pack_factor(n):
    """Chunks packed per class tile for a class with n nodes (4n rows).
    Must divide RING so groups never wrap the h ring."""
    if n == 0:
        return 0
    cap = 128 // (4 * n)
    for pf in (6, 4, 3, 2, 1):
        if pf <= cap:
            return pf
    return 1


# class codes: 0 = gauss, 1 = sin, 2 = tanh-class (tanh/sigmoid/identity)
def _cls_of_act(a):
    return {4: 0, 3: 1}.get(int(a), 2)


def _sorted_layout(act):
    """Order the H nodes by [gauss | sin | rest]; return (perm, n_gauss, n_sin).
    perm[j] = original node index placed at sorted slot j."""
    cls = np.array([_cls_of_act(a) for a in act])
    perm = np.argsort(cls, kind="stable")
    return perm, int((cls == 0).sum()), int((cls == 1).sum())


class _Plan:
    """Host-side folded weights + per-layer layouts. All float64 math."""

    def __init__(self, bias_in, W1, b1, act1, W2, b2, act2, W3, b3, act3,
                 Wout, bout):
        layers = [(W1, b1, act1), (W2, b2, act2), (W3, b3, act3)]
        self.perms, self.ngauss, self.nsin = [], [], []
        self.lhsT = []          # device stationary matrices (np.float32)
        self.cols = []          # per-layer dict of [128] operand columns
        # incoming per-node output transform: h_true = alpha*stored + beta
        in_alpha = np.ones(N_IN, dtype=np.float64)
        in_beta = np.asarray(bias_in, dtype=np.float64)  # h0 = x + bias_in
        in_dim = N_IN
        in_layout = None  # for L1 the input layout is the fixed feature order

        for li, (W, b, act) in enumerate(layers):
            W = np.asarray(W, dtype=np.float64)
            b = np.asarray(b, dtype=np.float64)
            act = np.asarray(act)
            perm, ng, ns = _sorted_layout(act)
            self.perms.append(perm)
            self.ngauss.append(ng)
            self.nsin.append(ns)

            # effective weights / bias absorbing incoming transforms
            W_eff = W * in_alpha[:, None]                  # [in_dim, H]
            b_eff = b + in_beta @ W                        # [H]

            # device stationary: block diagonal over groups with node sort
            K = G * in_dim
            lt = np.zeros((K, 128), dtype=np.float64)
            for g in range(G):
                for j in range(H):
                    node = perm[j]
                    m = self._row(li, g, j)
                    if li == 0:
                        rows = np.arange(in_dim) + in_dim * g
                        lt[rows, m] = W_eff[:, node]
                    else:
                        for k_in in range(in_dim):
                            kpart = in_layout[g][k_in]
                            lt[kpart, m] = W_eff[k_in, node]
            self.lhsT.append(lt.astype(np.float32))

            # operand columns.  Main tanh pass: per-partition scale/bias.
            tanh_scale = np.zeros(128, dtype=np.float64)
            tanh_bias = np.zeros(128, dtype=np.float64)
            out_alpha = np.ones(H, dtype=np.float64)
            out_beta = np.zeros(H, dtype=np.float64)
            for j in range(H):
                node = perm[j]
                a = int(act[node])
                be = b_eff[node]
                for g in range(G):
                    m = self._row(li, g, j)
                    if a == 1:        # tanh
                        tanh_scale[m] = 1.0
                        tanh_bias[m] = be
                    elif a == 2:      # sigmoid -> tanh(u/2)
                        tanh_scale[m] = 0.5
                        tanh_bias[m] = 0.5 * be
                    else:
                        # identity nodes AND the sin/gauss rows: the main
                        # tanh pass writes the identity-eps encoding
                        # tanh(eps*(u+b)) ~= eps*(u+b), which for sin/gauss
                        # is the value the packed chains gather from h
                        # (DMA cannot read PSUM).
                        tanh_scale[m] = float(ID_EPS)
                        tanh_bias[m] = float(ID_EPS) * be
                if a == 1:
                    out_alpha[node], out_beta[node] = 1.0, 0.0
                elif a == 2:
                    out_alpha[node], out_beta[node] = 0.5, 0.5
                elif a == 0:
                    out_alpha[node], out_beta[node] = 1.0 / float(ID_EPS), 0.0
                else:                 # sin / gauss: stored value is exact
                    out_alpha[node], out_beta[node] = 1.0, 0.0
            self.cols.append({
                "tanh_scale": tanh_scale, "tanh_bias": tanh_bias,
            })

            # next layer's incoming transform, in SORTED node order per device
            # partition -> but folds are per node; store per-node arrays and
            # the partition layout for the next lhsT build.
            in_alpha = out_alpha
            in_beta = out_beta
            in_dim = H
            # partition index of (g, sorted-slot j) for this layer's output
            in_layout = [[self._row(li, g, j) for j in range(H)]
                         for g in range(G)]
            # reorder alpha/beta to sorted-slot order for the next W_eff
            in_alpha = out_alpha[perm]
            in_beta = out_beta[perm]
            # next layer's W rows must be permuted accordingly
            if li < 2:
                layers[li + 1] = (np.asarray(layers[li + 1][0])[perm, :],
                                  layers[li + 1][1], layers[li + 1][2])
            else:
                self._wout_perm = perm

        # output layer
        Wo = np.asarray(Wout, dtype=np.float64)[self._wout_perm, :]
        bo = np.asarray(bout, dtype=np.float64)
        Wo_eff = Wo * in_alpha[:, None]
        bo_eff = bo + in_beta @ Wo
        lt = np.zeros((128, 32), dtype=np.float64)
        for g in range(G):
            for j in range(H):
                kpart = in_layout[g][j]
                for o in range(N_OUT):
                    lt[kpart, 3 * g + o] = Wo_eff[j, o]
        self.lhsT_out = lt.astype(np.float32)
        out_bias = np.zeros(128, dtype=np.float64)
        for q in range(4):
            for g in range(G):
                for o in range(N_OUT):
                    out_bias[32 * q + 3 * g + o] = bo_eff[o]
        self.out_bias = out_bias

        # pack all operand columns into one [128, 32] block
        colblk = np.zeros((128, 32), dtype=np.float64)
        for li in range(3):
            c = self.cols[li]
            colblk[:, 8 * li + 0] = c["tanh_scale"]
            colblk[:, 8 * li + 1] = c["tanh_bias"]
        colblk[:, 24] = self.out_bias
        self.colblk = colblk.astype(np.float32)

    @staticmethod
    def _row(li, g, j):
        """Device partition of sorted-slot j, group g (layer output layout).
        Rows are class-sorted ACROSS groups: slot j occupies partitions
        4*j + g."""
        return 4 * j + g


def _build_program(ngauss, nsin, p_core=P_CORE, chunk=CHUNK,
                   use_fp32r=True):
    """Build the bass module. Program structure depends only on the per-layer
    (n_gauss, n_sin) counts, not on weight values."""
    pg = p_core // G
    nchunk = pg // chunk
    nhalf = chunk // MM_N
    assert chunk % MM_N == 0 and pg % chunk == 0

    pfg = [_pack_factor(n) for n in ngauss]   # gauss pack factor per layer
    pfs = [_pack_factor(n) for n in nsin]     # sin pack factor per layer
    # emission skew between layers: covers each layer's largest pack-group
    # latency (a chunk's h completes only when its packed group completes;
    # skew is a priority hint, dataflow is dependency-enforced).  +1 for the
    # deferred chain phase B, +2 slack.
    s1 = max(pfg[0], pfs[0]) + 3
    s2 = s1 + max(pfg[1], pfs[1]) + 3
    s3 = s2 + max(pfg[2], pfs[2]) + 3
    skew = [0, s1, s2, s3]
    total_skew = skew[3] + 2

    nc = bacc.Bacc("TRN2", target_bir_lowering=False, debug=False,
                   num_devices=N_CORES)
    xT = nc.dram_tensor("xT", [G * N_IN, pg], F16, kind="ExternalInput").ap()
    wst = nc.dram_tensor("wst", [128, 416], F16, kind="ExternalInput").ap()
    cst = nc.dram_tensor("cst", [128, 64], F32, kind="ExternalInput").ap()
    yT = nc.dram_tensor("yT", [128, pg // 4], F32, kind="ExternalOutput").ap()

    with tile.TileContext(nc) as tc:
        cpool = tc.alloc_tile_pool(name="consts", bufs=1)
        wst_t = cpool.tile([128, 416], F16, tag="wst")
        cc_t = cpool.tile([128, 32], F32, tag="cc")
        nc.sync.dma_start(out=wst_t[:], in_=wst[:, 0:416])
        nc.sync.dma_start(out=cc_t[:], in_=cst[:, 0:32])
        w1_t = wst_t[:, 0:128]
        w2_t = wst_t[:, 128:256]
        w3_t = wst_t[:, 256:384]
        wo_t = wst_t[:, 384:416]
        col_t = cc_t[:, 0:32]

        ring = min(RING, nchunk)
        # persistent h ring tiles, one per hidden level (subtile-dep tracked)
        h1_t = cpool.tile([128, ring * chunk], F16, tag="h1")
        h2_t = cpool.tile([128, ring * chunk], F16, tag="h2")
        h3_t = cpool.tile([128, ring * chunk], F16, tag="h3")
        h_ring = [None, h1_t, h2_t, h3_t]

        # one SBUF work pool + one PSUM pool (per-tag bufs); fewer pools =
        # fewer release-barrier ceremonies in the teardown
        wpool = tc.alloc_tile_pool(name="work", bufs=2)
        xpool = gpool = spool = scpool = rpool = opool = wpool
        ppool = tc.alloc_tile_pool(name="psum", bufs=3, space="PSUM")
        oppool = ppool

        w_tiles = [w1_t, w2_t, w3_t]
        x_live = {}     # chunk -> x tile
        pso_live = {}   # chunk-pair -> psum_o tile
        def _new_cst():
            return {"pend": [], "subs": [], "row": 0, "npf": 0, "done": 0,
                    "tile": None}

        gst = {li: _new_cst() for li in range(3)}
        sst = {li: _new_cst() for li in range(3)}

        def hsl(c, n=1):
            """Column slice of n consecutive chunks starting at c (no wrap:
            pack factors divide the ring depth)."""
            r = c % ring
            assert r + n <= ring
            return slice(r * chunk, (r + n) * chunk)

        def emit_load(c):
            x_t = xpool.tile([G * N_IN, chunk], F16, tag="x", bufs=4)
            nc.gpsimd.dma_start(
                out=x_t[:], in_=xT[:, c * chunk:(c + 1) * chunk])
            x_live[c] = x_t

        def emit_main(c, li):
            """Main matmuls + full-height tanh pass."""
            if li == 0:
                h_prev = x_live.pop(c)
            else:
                h_prev = h_ring[li][:, hsl(c)]
            kdim = G * N_IN if li == 0 else 128
            ps = ppool.tile([128, chunk], F32, tag="pre")
            wt = w_tiles[li]
            for hh in range(nhalf):
                sl = slice(hh * MM_N, (hh + 1) * MM_N)
                nc.tensor.matmul(
                    ps[:, sl],
                    wt[0:kdim, :],
                    h_prev[0:kdim, sl],
                    start=True, stop=True,
                )
            cb = 8 * li
            # tanh-class pass over all 128 rows (junk eps-encode on the
            # gauss/sin rows, overwritten by the packed-chain scatters)
            nc.scalar.activation(
                h_ring[li + 1][:, hsl(c)], ps[:],
                mybir.ActivationFunctionType.Tanh,
                bias=col_t[:, cb + 1:cb + 2],
                scale=col_t[:, cb + 0:cb + 1],
            )
            # incremental sub-group gathers into the packed class tiles:
            # each group of npf chunks is gathered in two sub-DMAs (rows
            # [off : off + 4n*L] <- [4n, L*C], the reshape falls out of the
            # DMA flatten-order pairing), so the packed data is ready right
            # after the group's last main pass and every AP is a contiguous
            # row range (soundly dependency-tracked).
            ng, ns = ngauss[li], nsin[li]
            if ng > 0:
                self_gather(gst[li], li, c, pfg[li], nc.sync, gpool,
                            f"gz{li}", 0, 4 * ng)
            if ns > 0:
                self_gather(sst[li], li, c, pfs[li], nc.gpsimd, spool,
                            f"sz{li}", 4 * ng, 4 * ns)

        def self_gather(st, li, c, pf, eng, pool, tag, rbase, rows):
            """Append chunk c to the class group; gather a sub when half the
            group (or the remainder) has accumulated."""
            if st["tile"] is None:
                st["tile"] = pool.tile([128, chunk], F16, tag=tag, name=tag)
                st["npf"] = min(pf, nchunk - c)
                st["row"] = 0
                st["done"] = 0
                st["subs"] = []
                st["pend"] = []
            st["pend"].append(c)
            # sub lengths: ceil(npf/2) then the rest
            first = (st["npf"] + 1) // 2
            want = first if st["done"] == 0 else st["npf"] - first
            if len(st["pend"]) == want:
                L = want
                c_start = st["pend"][0]
                off = st["row"]
                eng.dma_start(
                    out=st["tile"][off:off + rows * L, :],
                    in_=h_ring[li + 1][rbase:rbase + rows, hsl(c_start, L)])
                st["subs"].append((off, c_start, L))
                st["row"] = off + rows * L
                st["done"] += L
                st["pend"] = []

        def scatter_subs(eng, res, li, rbase, rows, subs):
            for off, c_start, L in subs:
                eng.dma_start(
                    out=h_ring[li + 1][rbase:rbase + rows, hsl(c_start, L)],
                    in_=res[off:off + rows * L, :])

        def gauss_head(li):
            """Phase A: DVE square of the packed encodings (gathers already
            landed incrementally).  Returns phase-B state."""
            st = gst[li]
            R = st["row"]
            gz = st["tile"]
            subs = st["subs"]
            st["tile"] = None
            # y = enc^2 on DVE (fp32; the (0.25/eps^2) decode folds into the
            # Tanh scale) - keeps Square off the bottleneck ScalarE
            ysq = scpool.tile([128, chunk], F32, tag="gy", bufs=6)
            nc.vector.tensor_tensor(ysq[0:R, :], gz[0:R, :], gz[0:R, :],
                                    mybir.AluOpType.mult)
            return (li, ysq, subs, R)

        def gauss_act(st):
            """Phase B1: t = tanh(((u+b)/2)^2) on ScalarE."""
            li, ysq, subs, R = st
            t_t = scpool.tile([128, chunk], F32, tag="gt", bufs=3)
            nc.scalar.activation(
                t_t[0:R, :], ysq[0:R, :], mybir.ActivationFunctionType.Tanh,
                scale=float(0.25 / (ID_EPS * ID_EPS)),
            )
            return (li, t_t, subs, R)

        def gauss_tail(st):
            """Phase B2: den = 1 + t ; r = 1/den ; out = 2r - 1 =
            exp(-(z+b)^2/2); scatter the subs back."""
            li, t_t, subs, R = st
            # den = 1 + t, in place (DVE element-wise streaming)
            nc.vector.tensor_scalar(
                t_t[0:R, :], t_t[0:R, :], 1.0, None, mybir.AluOpType.add)
            rin_t = scpool.tile([128, chunk], F32, tag="gr", bufs=3)
            nc.vector.reciprocal_approx_fast(rin_t[0:R, :], t_t[0:R, :])
            g_r = rpool.tile([128, chunk], F16, tag="go", bufs=3)
            nc.vector.tensor_scalar(
                g_r[0:R, :], rin_t[0:R, :], 2.0, -1.0,
                mybir.AluOpType.mult, mybir.AluOpType.add)
            scatter_subs(nc.sync, g_r, li, 0, 4 * ngauss[li], subs)

        def sin_head(li):
            """Phase A: DVE range-wrap of the packed encodings in eps-space."""
            st = sst[li]
            R = st["row"]
            sz = st["tile"]
            subs = st["subs"]
            st["tile"] = None
            m_t = scpool.tile([128, chunk], F32, tag="sm", bufs=6)
            nc.vector.add_range_wrap(
                m_t[0:R, :], sz[0:R, :],
                0.0, float(ID_EPS) * PI, float(ID_EPS) * TWO_PI)
            return (li, m_t, subs, R)

        def sin_act(st):
            """Phase B: Sin decodes with scale 1/eps; scatter the subs."""
            li, m_t, subs, R = st
            s_r = rpool.tile([128, chunk], F16, tag="so", bufs=3)
            nc.scalar.activation(
                s_r[0:R, :], m_t[0:R, :], mybir.ActivationFunctionType.Sin,
                scale=float(1.0 / ID_EPS))
            scatter_subs(nc.gpsimd, s_r, li, 4 * ngauss[li], 4 * nsin[li],
                         subs)

        def collect_flushes(c, li, heads):
            """After emit_main(c, li): start phase A for completed groups."""
            if ngauss[li] > 0 and gst[li]["tile"] is not None \
                    and gst[li]["done"] == gst[li]["npf"]:
                heads.append(("g", gauss_head(li)))
            if nsin[li] > 0 and sst[li]["tile"] is not None \
                    and sst[li]["done"] == sst[li]["npf"]:
                heads.append(("s", sin_head(li)))

        def emit_out(c):
            # output layer: quadrant-packed [12,512] matmuls
            h_prev = h_ring[3][:, hsl(c)]
            q0 = 2 * (c % 2)
            if q0 == 0:
                pso_live[c // 2] = oppool.tile([128, MM_N], F32, tag="preo",
                                               name="pso", bufs=2)
            pso = pso_live[c // 2]
            for hh in range(nhalf):
                q = q0 + hh
                nc.tensor.matmul(
                    pso[32 * q:32 * q + 32, :],
                    wo_t,
                    h_prev[:, hh * MM_N:(hh + 1) * MM_N],
                    start=True, stop=True,
                    tile_position=(0, 32 * q),
                )
            if q0 == 2:
                pso_live.pop(c // 2)
                osb = opool.tile([128, MM_N], F32, tag="osb")
                nc.scalar.activation(
                    osb[:], pso[:],
                    mybir.ActivationFunctionType.Tanh,
                    bias=col_t[:, 24:25],
                )
                # one full-height store per chunk-pair; host unpacks the
                # quadrant layout (junk rows included)
                k = c // 2
                nc.sync.dma_start(
                    out=yT[:, k * MM_N:(k + 1) * MM_N], in_=osb[:])

        # Software-pipelined emission with per-layer skew covering the packed
        # group latency.  Step order [L2, L3, L1, out] keeps the out-layer
        # matmuls (which wait on L3 chain scatters) from head-of-line
        # blocking L1's matmuls in the PE FIFO, and defers each chain's
        # ScalarE op (phase B) one full step behind its DVE head (phase A)
        # so it is ready when the ACT queue reaches it.
        assert nchunk % 2 == 0
        emit_load(0)
        pending = []   # phase-A states from the previous step
        for t in range(nchunk + total_skew):
            heads = []
            if skew[1] <= t and t - skew[1] < nchunk:
                emit_main(t - skew[1], 1)
                collect_flushes(t - skew[1], 1, heads)
            if skew[2] <= t and t - skew[2] < nchunk:
                emit_main(t - skew[2], 2)
                collect_flushes(t - skew[2], 2, heads)
            if t + 1 < nchunk:
                emit_load(t + 1)
            if t < nchunk:
                emit_main(t, 0)
                collect_flushes(t, 0, heads)
            if skew[3] <= t and t - skew[3] < nchunk:
                emit_out(t - skew[3])
            # phase B for last step's groups: ACT parts first, tails after
            bstates = []
            for kind, st in pending:
                bstates.append((kind, gauss_act(st) if kind == "g" else st))
            for kind, st in bstates:
                if kind == "g":
                    gauss_tail(st)
                else:
                    sin_act(st)
            pending = heads

        for p in (ppool, wpool, cpool):
            p.release()

    nc.compile()
    return nc


_PROGRAM_CACHE = {}


def _get_program(ngauss, nsin, p_core=P_CORE, chunk=CHUNK, use_fp32r=True):
    key = (tuple(ngauss), tuple(nsin), p_core, chunk, use_fp32r)
    if key not in _PROGRAM_CACHE:
        _PROGRAM_CACHE[key] = _build_program(ngauss, nsin, p_core, chunk,
                                             use_fp32r=use_fp32r)
    return _PROGRAM_CACHE[key]


def make_in_maps(inputs, plan, p_core=P_CORE, n_cores=N_CORES):
    """Shard + transpose the pixel data; replicate constants."""
    x = np.ascontiguousarray(np.asarray(inputs["inputs"], dtype=np.float32))
    pg = p_core // G
    in_maps = []
    for core in range(n_cores):
        xc = x[core * p_core:(core + 1) * p_core]          # [p_core, 12]
        xg = xc.reshape(G, pg, N_IN)                        # [G, pg, 12]
        xT = np.ascontiguousarray(xg.transpose(0, 2, 1)
                                  .reshape(G * N_IN, pg)
                                  .astype(np.float16))      # [48, pg]
        wst = np.zeros((128, 416), dtype=np.float16)
        wst[0:G * N_IN, 0:128] = plan.lhsT[0]
        wst[:, 128:256] = plan.lhsT[1]
        wst[:, 256:384] = plan.lhsT[2]
        wst[:, 384:416] = plan.lhsT_out
        cst = np.zeros((128, 64), dtype=np.float32)
        cst[:, 0:32] = plan.colblk
        in_maps.append({"xT": xT, "wst": wst, "cst": cst})
    return in_maps


def assemble_output(results, p_core=P_CORE, n_cores=N_CORES):
    pg = p_core // G
    nk = pg // (2 * CHUNK) if pg >= 2 * CHUNK else 1
    out = np.empty((p_core * n_cores, N_OUT), dtype=np.float32)
    for core in range(n_cores):
        yT = results[core]["yT"]                     # [128, pg/4]
        # quadrant layout: yT[32q + 3g + o, 512k + j] is (g, o) of slot
        # s = 2048k + 512q + j
        arr = yT.reshape(4, 32, nk, MM_N)            # [q, row, k, j]
        yc = (arr[:, 0:12]                           # [q, 3g+o, k, j]
              .transpose(1, 2, 0, 3)                 # [3g+o, k, q, j]
              .reshape(G, N_OUT, pg)                 # [g, o, slot]
              .transpose(0, 2, 1))                   # [g, slot, o]
        out[core * p_core:(core + 1) * p_core] = yc.reshape(p_core, N_OUT)
    return out


def make_plan(inputs):
    return _Plan(
        inputs["bias_in"], inputs["W1"], inputs["b1"], inputs["act1"],
        inputs["W2"], inputs["b2"], inputs["act2"],
        inputs["W3"], inputs["b3"], inputs["act3"],
        inputs["Wout"], inputs["bout"])


def run(inputs, trace=False, use_fp32r=True, **spmd_kwargs):
    plan = make_plan(inputs)
    nc = _get_program(plan.ngauss, plan.nsin, use_fp32r=use_fp32r)
    in_maps = make_in_maps(inputs, plan)
    res = run_bass_kernel_spmd(nc, in_maps, list(range(N_CORES)),
                               trace=trace, **spmd_kwargs)
    return assemble_output(res.results), res


def kernel(**inputs) -> np.ndarray:
    out, _ = run(inputs, trace=False)
    return out


# revision 33
# speedup vs baseline: 1.0304x; 1.0105x over previous
"""CPPN MLP (12 -> 32 -> 32 -> 32 -> 3, per-node activations) on 8 TRN2 cores.

Data-parallel over the pixel axis. Each core processes P_CORE pixels laid out
feature-major as 4 pixel-groups on SBUF partitions:
  rhs partition (12*g + i) holds feature i of pixel-group g  (layer-1 input)
  hidden state partition layout per layer: 4 groups x 32 nodes, nodes sorted
  [gauss | sin | tanh-class] across groups.

All matmul data (x, weights, hidden state h) is fp16: full-rate PE matmuls,
half the DMA bytes, 10-bit mantissa (~5e-4 relative) which the 2e-2 harness
gate easily absorbs.  PSUM accumulation stays fp32.

The tanh-class (tanh/sigmoid/identity) is handled by ONE Tanh pass over all
128 partitions with per-partition scale/bias operands plus host-side
algebraic folds into the next layer's weights:
  sigmoid(z) = 0.5*tanh(z/2) + 0.5          (stored tanh(z/2); affine folded)
  identity(z) = tanh(eps*z)/eps             (stored tanh(eps*z); 1/eps folded)

Sin and gauss rows are only a fraction of the partitions but a sub-range
activation op costs the same as a full-height one (cost ~ free-dim length).
So the main Tanh pass writes those rows as the identity-eps encoding
tanh(eps*(u+b)) ~= eps*(u+b) (eps = 2^-9).  Each hidden level h lives in ONE
persistent ring tile [128, RING*CHUNK], so a whole pack-group of chunks is a
contiguous column slice: the class rows of pf consecutive chunks are moved
into a densely packed tile [4*n*pf, CHUNK] with a SINGLE gather DMA (the
partition/free reshape falls out of DMA flatten-order pairing), the per-class
op chains run once per packed tile (amortized pf-fold), and one scatter DMA
writes the results back over the eps-junk rows of all pf chunks:
  gauss(z) = exp(-(z+b)^2/2):  DVE squares the encoding (y = enc^2, fp32);
    t = Tanh((0.25/eps^2)*y) = tanh(((u+b)/2)^2);  gauss = 2/(1+t) - 1 via
    DVE add + reciprocal_approx_fast + one affine tensor_scalar (also the
    fp32->fp16 convert).  Square runs on DVE, not ScalarE: ScalarE is the
    bottleneck engine (3 main passes + sin + gauss tanh + out tanh).
  sin(z+b):  ADD_RANGE_WRAP wraps the encoding into [-eps*pi, eps*pi] (the
    wrap is linear so it works in eps-space; one period suffices since
    |z+b| < 3*pi), then Sin decodes with scale 1/eps.
Pack factors are divisors of RING so a group never wraps the ring.  Junk rows
above the packed region flow through every op harmlessly.  The output stores
write the full [128, 512] quadrant-packed tile per chunk-pair (junk rows
included) so each is ONE descriptor; the host unpacks.  DMA issue is spread
across sequencers (gauss gathers/scatters + output stores on SP's DGE; x
loads and sin gathers/scatters on GpSimd's; none on Activation) so no
sequencer's DIRECT2D issue cost (~0.7us each) starves ScalarE.
"""

import os
import sys

import numpy as np

_REPO = "/root/.axon_site/_ro/trn_rl_repo"
if _REPO not in sys.path and not os.path.isdir("/opt/trn_rl_repo"):
    sys.path.insert(0, _REPO)

import concourse.bacc as bacc
import concourse.bass as bass  # noqa: F401
import concourse.tile as tile
from concourse import mybir
from concourse.bass_utils import run_bass_kernel_spmd

# Pin the activation-function table to the single set containing every
# function this kernel uses ({Tanh, Sin}).  Without this, bacc's greedy
# per-instruction set selection can alternate between sets and emit an
# ACT_TABLE_LOAD (~2.7us) per chunk.
_orig_get_tables = bacc.get_activation_tables


def _pinned_tables(arch):
    t = _orig_get_tables(arch)
    if "silu_and_others" in t:
        # act_func_set_id is the POSITION in act_info.json's set list, so
        # keep every entry (order intact) and just empty the others.
        return {name: (funcs if name == "silu_and_others" else set())
                for name, funcs in t.items()}
    return t


bacc.get_activation_tables = _pinned_tables

F32 = mybir.dt.float32
F16 = mybir.dt.float16

P_TOTAL = 1024 * 1024
N_IN, H, N_OUT = 12, 32, 3
N_CORES = 8
P_CORE = P_TOTAL // N_CORES  # 131072
G = 4                        # pixel groups packed on partitions
PG = P_CORE // G             # 32768 pixels per group per core
CHUNK = 1024                 # pixels per group per chunk (2 PSUM banks)
MM_N = 512                   # matmul moving free dim (one PSUM bank)
RING = 12                    # h ring depth (chunks) per hidden level
ID_EPS = np.float32(2.0 ** -9)      # identity-via-tanh input scale
TWO_PI = float(2.0 * np.pi)
PI = float(np.pi)


def _pack_factor(n):
    """Chunks packed per class tile for a class with n nodes (4n rows).
    Must divide RING so groups never wrap the h ring."""
    if n == 0:
        return 0
    cap = 128 // (4 * n)
    for pf in (6, 4, 3, 2, 1):
        if pf <= cap:
            return pf
    return 1


# class codes: 0 = gauss, 1 = sin, 2 = tanh-class (tanh/sigmoid/identity)
def _cls_of_act(a):
    return {4: 0, 3: 1}.get(int(a), 2)


def _sorted_layout(act):
    """Order the H nodes by [gauss | sin | rest]; return (perm, n_gauss, n_sin).
    perm[j] = original node index placed at sorted slot j."""
    cls = np.array([_cls_of_act(a) for a in act])
    perm = np.argsort(cls, kind="stable")
    return perm, int((cls == 0).sum()), int((cls == 1).sum())


class _Plan:
    """Host-side folded weights + per-layer layouts. All float64 math."""

    def __init__(self, bias_in, W1, b1, act1, W2, b2, act2, W3, b3, act3,
                 Wout, bout):
        layers = [(W1, b1, act1), (W2, b2, act2), (W3, b3, act3)]
        self.perms, self.ngauss, self.nsin = [], [], []
        self.lhsT = []          # device stationary matrices (np.float32)
        self.cols = []          # per-layer dict of [128] operand columns
        # incoming per-node output transform: h_true = alpha*stored + beta
        in_alpha = np.ones(N_IN, dtype=np.float64)
        in_beta = np.asarray(bias_in, dtype=np.float64)  # h0 = x + bias_in
        in_dim = N_IN
        in_layout = None  # for L1 the input layout is the fixed feature order

        for li, (W, b, act) in enumerate(layers):
            W = np.asarray(W, dtype=np.float64)
            b = np.asarray(b, dtype=np.float64)
            act = np.asarray(act)
            perm, ng, ns = _sorted_layout(act)
            self.perms.append(perm)
            self.ngauss.append(ng)
            self.nsin.append(ns)

            # effective weights / bias absorbing incoming transforms
            W_eff = W * in_alpha[:, None]                  # [in_dim, H]
            b_eff = b + in_beta @ W                        # [H]

            # device stationary: block diagonal over groups with node sort
            K = G * in_dim
            lt = np.zeros((K, 128), dtype=np.float64)
            for g in range(G):
                for j in range(H):
                    node = perm[j]
                    m = self._row(li, g, j)
                    if li == 0:
                        rows = np.arange(in_dim) + in_dim * g
                        lt[rows, m] = W_eff[:, node]
                    else:
                        for k_in in range(in_dim):
                            kpart = in_layout[g][k_in]
                            lt[kpart, m] = W_eff[k_in, node]
            self.lhsT.append(lt.astype(np.float32))

            # operand columns.  Main tanh pass: per-partition scale/bias.
            tanh_scale = np.zeros(128, dtype=np.float64)
            tanh_bias = np.zeros(128, dtype=np.float64)
            out_alpha = np.ones(H, dtype=np.float64)
            out_beta = np.zeros(H, dtype=np.float64)
            for j in range(H):
                node = perm[j]
                a = int(act[node])
                be = b_eff[node]
                for g in range(G):
                    m = self._row(li, g, j)
                    if a == 1:        # tanh
                        tanh_scale[m] = 1.0
                        tanh_bias[m] = be
                    elif a == 2:      # sigmoid -> tanh(u/2)
                        tanh_scale[m] = 0.5
                        tanh_bias[m] = 0.5 * be
                    else:
                        # identity nodes AND the sin/gauss rows: the main
                        # tanh pass writes the identity-eps encoding
                        # tanh(eps*(u+b)) ~= eps*(u+b), which for sin/gauss
                        # is the value the packed chains gather from h
                        # (DMA cannot read PSUM).
                        tanh_scale[m] = float(ID_EPS)
                        tanh_bias[m] = float(ID_EPS) * be
                if a == 1:
                    out_alpha[node], out_beta[node] = 1.0, 0.0
                elif a == 2:
                    out_alpha[node], out_beta[node] = 0.5, 0.5
                elif a == 0:
                    out_alpha[node], out_beta[node] = 1.0 / float(ID_EPS), 0.0
                else:                 # sin / gauss: stored value is exact
                    out_alpha[node], out_beta[node] = 1.0, 0.0
            self.cols.append({
                "tanh_scale": tanh_scale, "tanh_bias": tanh_bias,
            })

            # next layer's incoming transform, in SORTED node order per device
            # partition -> but folds are per node; store per-node arrays and
            # the partition layout for the next lhsT build.
            in_alpha = out_alpha
            in_beta = out_beta
            in_dim = H
            # partition index of (g, sorted-slot j) for this layer's output
            in_layout = [[self._row(li, g, j) for j in range(H)]
                         for g in range(G)]
            # reorder alpha/beta to sorted-slot order for the next W_eff
            in_alpha = out_alpha[perm]
            in_beta = out_beta[perm]
            # next layer's W rows must be permuted accordingly
            if li < 2:
                layers[li + 1] = (np.asarray(layers[li + 1][0])[perm, :],
                                  layers[li + 1][1], layers[li + 1][2])
            else:
                self._wout_perm = perm

        # output layer
        Wo = np.asarray(Wout, dtype=np.float64)[self._wout_perm, :]
        bo = np.asarray(bout, dtype=np.float64)
        Wo_eff = Wo * in_alpha[:, None]
        bo_eff = bo + in_beta @ Wo
        lt = np.zeros((128, 32), dtype=np.float64)
        for g in range(G):
            for j in range(H):
                kpart = in_layout[g][j]
                for o in range(N_OUT):
                    lt[kpart, 3 * g + o] = Wo_eff[j, o]
        self.lhsT_out = lt.astype(np.float32)
        out_bias = np.zeros(128, dtype=np.float64)
        for q in range(4):
            for g in range(G):
                for o in range(N_OUT):
                    out_bias[32 * q + 3 * g + o] = bo_eff[o]
        self.out_bias = out_bias

        # pack all operand columns into one [128, 32] block
        colblk = np.zeros((128, 32), dtype=np.float64)
        for li in range(3):
            c = self.cols[li]
            colblk[:, 8 * li + 0] = c["tanh_scale"]
            colblk[:, 8 * li + 1] = c["tanh_bias"]
        colblk[:, 24] = self.out_bias
        self.colblk = colblk.astype(np.float32)

    @staticmethod
    def _row(li, g, j):
        """Device partition of sorted-slot j, group g (layer output layout).
        Rows are class-sorted ACROSS groups: slot j occupies partitions
        4*j + g."""
        return 4 * j + g


def _build_program(ngauss, nsin, p_core=P_CORE, chunk=CHUNK,
                   use_fp32r=True):
    """Build the bass module. Program structure depends only on the per-layer
    (n_gauss, n_sin) counts, not on weight values."""
    pg = p_core // G
    nchunk = pg // chunk
    nhalf = chunk // MM_N
    assert chunk % MM_N == 0 and pg % chunk == 0

    pfg = [_pack_factor(n) for n in ngauss]   # gauss pack factor per layer
    pfs = [_pack_factor(n) for n in nsin]     # sin pack factor per layer
    # emission skew between layers: covers each layer's largest pack-group
    # latency (a chunk's h completes only when its packed group completes;
    # skew is a priority hint, dataflow is dependency-enforced).  +1 for the
    # deferred chain phase B, +2 slack.
    s1 = max(pfg[0], pfs[0]) + 3
    s2 = s1 + max(pfg[1], pfs[1]) + 3
    s3 = s2 + max(pfg[2], pfs[2]) + 3
    skew = [0, s1, s2, s3]
    total_skew = skew[3] + 2

    nc = bacc.Bacc("TRN2", target_bir_lowering=False, debug=False,
                   num_devices=N_CORES)
    xT = nc.dram_tensor("xT", [G * N_IN, pg], F16, kind="ExternalInput").ap()
    wst = nc.dram_tensor("wst", [128, 416], F16, kind="ExternalInput").ap()
    cst = nc.dram_tensor("cst", [128, 64], F32, kind="ExternalInput").ap()
    yT = nc.dram_tensor("yT", [128, pg // 4], F32, kind="ExternalOutput").ap()

    with tile.TileContext(nc) as tc:
        cpool = tc.alloc_tile_pool(name="consts", bufs=1)
        wst_t = cpool.tile([128, 416], F16, tag="wst")
        cc_t = cpool.tile([128, 32], F32, tag="cc")
        nc.sync.dma_start(out=wst_t[:, 0:128], in_=wst[:, 0:128])
        nc.sync.dma_start(out=cc_t[:], in_=cst[:, 0:32])
        nc.sync.dma_start(out=wst_t[:, 128:416], in_=wst[:, 128:416])
        w1_t = wst_t[:, 0:128]
        w2_t = wst_t[:, 128:256]
        w3_t = wst_t[:, 256:384]
        wo_t = wst_t[:, 384:416]
        col_t = cc_t[:, 0:32]

        ring = min(RING, nchunk)
        # persistent h ring tiles, one per hidden level (subtile-dep tracked)
        h1_t = cpool.tile([128, ring * chunk], F16, tag="h1")
        h2_t = cpool.tile([128, ring * chunk], F16, tag="h2")
        h3_t = cpool.tile([128, ring * chunk], F16, tag="h3")
        h_ring = [None, h1_t, h2_t, h3_t]

        # one SBUF work pool + one PSUM pool (per-tag bufs); fewer pools =
        # fewer release-barrier ceremonies in the teardown
        wpool = tc.alloc_tile_pool(name="work", bufs=2)
        xpool = gpool = spool = scpool = rpool = opool = wpool
        ppool = tc.alloc_tile_pool(name="psum", bufs=3, space="PSUM")
        oppool = ppool

        w_tiles = [w1_t, w2_t, w3_t]
        x_live = {}     # chunk -> x tile
        pso_live = {}   # chunk-pair -> psum_o tile
        def _new_cst():
            return {"pend": [], "subs": [], "row": 0, "npf": 0, "done": 0,
                    "tile": None}

        gst = {li: _new_cst() for li in range(3)}
        sst = {li: _new_cst() for li in range(3)}

        def hsl(c, n=1):
            """Column slice of n consecutive chunks starting at c (no wrap:
            pack factors divide the ring depth)."""
            r = c % ring
            assert r + n <= ring
            return slice(r * chunk, (r + n) * chunk)

        def emit_load(c):
            x_t = xpool.tile([G * N_IN, chunk], F16, tag="x", bufs=4)
            nc.gpsimd.dma_start(
                out=x_t[:], in_=xT[:, c * chunk:(c + 1) * chunk])
            x_live[c] = x_t

        def emit_main(c, li):
            """Main matmuls + full-height tanh pass."""
            if li == 0:
                h_prev = x_live.pop(c)
            else:
                h_prev = h_ring[li][:, hsl(c)]
            kdim = G * N_IN if li == 0 else 128
            ps = ppool.tile([128, chunk], F32, tag="pre")
            wt = w_tiles[li]
            for hh in range(nhalf):
                sl = slice(hh * MM_N, (hh + 1) * MM_N)
                nc.tensor.matmul(
                    ps[:, sl],
                    wt[0:kdim, :],
                    h_prev[0:kdim, sl],
                    start=True, stop=True,
                )
            cb = 8 * li
            # tanh-class pass over all 128 rows (junk eps-encode on the
            # gauss/sin rows, overwritten by the packed-chain scatters)
            nc.scalar.activation(
                h_ring[li + 1][:, hsl(c)], ps[:],
                mybir.ActivationFunctionType.Tanh,
                bias=col_t[:, cb + 1:cb + 2],
                scale=col_t[:, cb + 0:cb + 1],
            )
            # incremental sub-group gathers into the packed class tiles:
            # each group of npf chunks is gathered in two sub-DMAs (rows
            # [off : off + 4n*L] <- [4n, L*C], the reshape falls out of the
            # DMA flatten-order pairing), so the packed data is ready right
            # after the group's last main pass and every AP is a contiguous
            # row range (soundly dependency-tracked).
            ng, ns = ngauss[li], nsin[li]
            if ng > 0:
                self_gather(gst[li], li, c, pfg[li], nc.sync, gpool,
                            f"gz{li}", 0, 4 * ng)
            if ns > 0:
                self_gather(sst[li], li, c, pfs[li], nc.gpsimd, spool,
                            f"sz{li}", 4 * ng, 4 * ns)

        def self_gather(st, li, c, pf, eng, pool, tag, rbase, rows):
            """Append chunk c to the class group; gather a sub when half the
            group (or the remainder) has accumulated."""
            if st["tile"] is None:
                st["tile"] = pool.tile([128, chunk], F16, tag=tag, name=tag)
                st["npf"] = min(pf, nchunk - c)
                st["row"] = 0
                st["done"] = 0
                st["subs"] = []
                st["pend"] = []
            st["pend"].append(c)
            # sub lengths: ceil(npf/2) then the rest
            first = (st["npf"] + 1) // 2
            want = first if st["done"] == 0 else st["npf"] - first
            if len(st["pend"]) == want:
                L = want
                c_start = st["pend"][0]
                off = st["row"]
                eng.dma_start(
                    out=st["tile"][off:off + rows * L, :],
                    in_=h_ring[li + 1][rbase:rbase + rows, hsl(c_start, L)])
                st["subs"].append((off, c_start, L))
                st["row"] = off + rows * L
                st["done"] += L
                st["pend"] = []

        def scatter_subs(eng, res, li, rbase, rows, subs):
            for off, c_start, L in subs:
                eng.dma_start(
                    out=h_ring[li + 1][rbase:rbase + rows, hsl(c_start, L)],
                    in_=res[off:off + rows * L, :])

        def gauss_head(li):
            """Phase A: DVE square of the packed encodings (gathers already
            landed incrementally).  Returns phase-B state."""
            st = gst[li]
            R = st["row"]
            gz = st["tile"]
            subs = st["subs"]
            st["tile"] = None
            # y = enc^2 on DVE (fp32; the (0.25/eps^2) decode folds into the
            # Tanh scale) - keeps Square off the bottleneck ScalarE
            ysq = scpool.tile([128, chunk], F32, tag="gy", bufs=6)
            nc.vector.tensor_tensor(ysq[0:R, :], gz[0:R, :], gz[0:R, :],
                                    mybir.AluOpType.mult)
            return (li, ysq, subs, R)

        def gauss_act(st):
            """Phase B1: t = tanh(((u+b)/2)^2) on ScalarE."""
            li, ysq, subs, R = st
            t_t = scpool.tile([128, chunk], F32, tag="gt", bufs=3)
            nc.scalar.activation(
                t_t[0:R, :], ysq[0:R, :], mybir.ActivationFunctionType.Tanh,
                scale=float(0.25 / (ID_EPS * ID_EPS)),
            )
            return (li, t_t, subs, R)

        def gauss_tail(st):
            """Phase B2: den = 1 + t ; r = 1/den ; out = 2r - 1 =
            exp(-(z+b)^2/2); scatter the subs back."""
            li, t_t, subs, R = st
            # den = 1 + t, in place (DVE element-wise streaming)
            nc.vector.tensor_scalar(
                t_t[0:R, :], t_t[0:R, :], 1.0, None, mybir.AluOpType.add)
            rin_t = scpool.tile([128, chunk], F32, tag="gr", bufs=3)
            nc.vector.reciprocal_approx_fast(rin_t[0:R, :], t_t[0:R, :])
            g_r = rpool.tile([128, chunk], F16, tag="go", bufs=3)
            nc.vector.tensor_scalar(
                g_r[0:R, :], rin_t[0:R, :], 2.0, -1.0,
                mybir.AluOpType.mult, mybir.AluOpType.add)
            scatter_subs(nc.sync, g_r, li, 0, 4 * ngauss[li], subs)

        def sin_head(li):
            """Phase A: DVE range-wrap of the packed encodings in eps-space."""
            st = sst[li]
            R = st["row"]
            sz = st["tile"]
            subs = st["subs"]
            st["tile"] = None
            m_t = scpool.tile([128, chunk], F32, tag="sm", bufs=6)
            nc.vector.add_range_wrap(
                m_t[0:R, :], sz[0:R, :],
                0.0, float(ID_EPS) * PI, float(ID_EPS) * TWO_PI)
            return (li, m_t, subs, R)

        def sin_act(st):
            """Phase B: Sin decodes with scale 1/eps; scatter the subs."""
            li, m_t, subs, R = st
            s_r = rpool.tile([128, chunk], F16, tag="so", bufs=3)
            nc.scalar.activation(
                s_r[0:R, :], m_t[0:R, :], mybir.ActivationFunctionType.Sin,
                scale=float(1.0 / ID_EPS))
            scatter_subs(nc.gpsimd, s_r, li, 4 * ngauss[li], 4 * nsin[li],
                         subs)

        def collect_flushes(c, li, heads):
            """After emit_main(c, li): start phase A for completed groups."""
            if ngauss[li] > 0 and gst[li]["tile"] is not None \
                    and gst[li]["done"] == gst[li]["npf"]:
                heads.append(("g", gauss_head(li)))
            if nsin[li] > 0 and sst[li]["tile"] is not None \
                    and sst[li]["done"] == sst[li]["npf"]:
                heads.append(("s", sin_head(li)))

        def emit_out(c):
            # output layer: quadrant-packed [12,512] matmuls into a quad
            # psum tile [128, 1024] (2 banks); one Tanh pass + one store per
            # 4 chunks (flat yT layout identical to the per-pair variant)
            h_prev = h_ring[3][:, hsl(c)]
            q0 = 2 * (c % 2)
            off = ((c % 4) // 2) * MM_N
            if c % 4 == 0:
                pso_live[c // 4] = oppool.tile([128, 2 * MM_N], F32,
                                               tag="preo", name="pso",
                                               bufs=1)
            pso = pso_live[c // 4]
            for hh in range(nhalf):
                q = q0 + hh
                nc.tensor.matmul(
                    pso[32 * q:32 * q + 32, off:off + MM_N],
                    wo_t,
                    h_prev[:, hh * MM_N:(hh + 1) * MM_N],
                    start=True, stop=True,
                    tile_position=(0, 32 * q),
                )
            if c % 4 == 3 or c == nchunk - 1:
                pso_live.pop(c // 4)
                ncols = off + MM_N
                osb = opool.tile([128, 2 * MM_N], F32, tag="osb")
                nc.scalar.activation(
                    osb[:, 0:ncols], pso[:, 0:ncols],
                    mybir.ActivationFunctionType.Tanh,
                    bias=col_t[:, 24:25],
                )
                k = c // 4
                nc.sync.dma_start(
                    out=yT[:, k * 2 * MM_N:k * 2 * MM_N + ncols],
                    in_=osb[:, 0:ncols])

        # Software-pipelined emission with per-layer skew covering the packed
        # group latency.  Step order [L2, L3, L1, out] keeps the out-layer
        # matmuls (which wait on L3 chain scatters) from head-of-line
        # blocking L1's matmuls in the PE FIFO, and defers each chain's
        # ScalarE op (phase B) one full step behind its DVE head (phase A)
        # so it is ready when the ACT queue reaches it.
        assert nchunk % 2 == 0
        emit_load(0)
        pending = []   # phase-A states from the previous step
        for t in range(nchunk + total_skew):
            heads = []
            if skew[1] <= t and t - skew[1] < nchunk:
                emit_main(t - skew[1], 1)
                collect_flushes(t - skew[1], 1, heads)
            if skew[2] <= t and t - skew[2] < nchunk:
                emit_main(t - skew[2], 2)
                collect_flushes(t - skew[2], 2, heads)
            if t + 1 < nchunk:
                emit_load(t + 1)
            if t < nchunk:
                emit_main(t, 0)
                collect_flushes(t, 0, heads)
            if skew[3] <= t and t - skew[3] < nchunk:
                emit_out(t - skew[3])
            # phase B for last step's groups: ACT parts first, tails after
            bstates = []
            for kind, st in pending:
                bstates.append((kind, gauss_act(st) if kind == "g" else st))
            for kind, st in bstates:
                if kind == "g":
                    gauss_tail(st)
                else:
                    sin_act(st)
            pending = heads

        for p in (ppool, wpool, cpool):
            p.release()

    nc.compile()
    return nc


_PROGRAM_CACHE = {}


def _get_program(ngauss, nsin, p_core=P_CORE, chunk=CHUNK, use_fp32r=True):
    key = (tuple(ngauss), tuple(nsin), p_core, chunk, use_fp32r)
    if key not in _PROGRAM_CACHE:
        _PROGRAM_CACHE[key] = _build_program(ngauss, nsin, p_core, chunk,
                                             use_fp32r=use_fp32r)
    return _PROGRAM_CACHE[key]


def make_in_maps(inputs, plan, p_core=P_CORE, n_cores=N_CORES):
    """Shard + transpose the pixel data; replicate constants."""
    x = np.ascontiguousarray(np.asarray(inputs["inputs"], dtype=np.float32))
    pg = p_core // G
    in_maps = []
    for core in range(n_cores):
        xc = x[core * p_core:(core + 1) * p_core]          # [p_core, 12]
        xg = xc.reshape(G, pg, N_IN)                        # [G, pg, 12]
        xT = np.ascontiguousarray(xg.transpose(0, 2, 1)
                                  .reshape(G * N_IN, pg)
                                  .astype(np.float16))      # [48, pg]
        wst = np.zeros((128, 416), dtype=np.float16)
        wst[0:G * N_IN, 0:128] = plan.lhsT[0]
        wst[:, 128:256] = plan.lhsT[1]
        wst[:, 256:384] = plan.lhsT[2]
        wst[:, 384:416] = plan.lhsT_out
        cst = np.zeros((128, 64), dtype=np.float32)
        cst[:, 0:32] = plan.colblk
        in_maps.append({"xT": xT, "wst": wst, "cst": cst})
    return in_maps


def assemble_output(results, p_core=P_CORE, n_cores=N_CORES):
    pg = p_core // G
    nk = pg // (2 * CHUNK) if pg >= 2 * CHUNK else 1
    out = np.empty((p_core * n_cores, N_OUT), dtype=np.float32)
    for core in range(n_cores):
        yT = results[core]["yT"]                     # [128, pg/4]
        # quadrant layout: yT[32q + 3g + o, 512k + j] is (g, o) of slot
        # s = 2048k + 512q + j
        arr = yT.reshape(4, 32, nk, MM_N)            # [q, row, k, j]
        yc = (arr[:, 0:12]                           # [q, 3g+o, k, j]
              .transpose(1, 2, 0, 3)                 # [3g+o, k, q, j]
              .reshape(G, N_OUT, pg)                 # [g, o, slot]
              .transpose(0, 2, 1))                   # [g, slot, o]
        out[core * p_core:(core + 1) * p_core] = yc.reshape(p_core, N_OUT)
    return out


def make_plan(inputs):
    return _Plan(
        inputs["bias_in"], inputs["W1"], inputs["b1"], inputs["act1"],
        inputs["W2"], inputs["b2"], inputs["act2"],
        inputs["W3"], inputs["b3"], inputs["act3"],
        inputs["Wout"], inputs["bout"])


def run(inputs, trace=False, use_fp32r=True, **spmd_kwargs):
    plan = make_plan(inputs)
    nc = _get_program(plan.ngauss, plan.nsin, use_fp32r=use_fp32r)
    in_maps = make_in_maps(inputs, plan)
    res = run_bass_kernel_spmd(nc, in_maps, list(range(N_CORES)),
                               trace=trace, **spmd_kwargs)
    return assemble_output(res.results), res


def kernel(**inputs) -> np.ndarray:
    out, _ = run(inputs, trace=False)
    return out


# revision 34
# speedup vs baseline: 1.0580x; 1.0268x over previous
"""CPPN MLP (12 -> 32 -> 32 -> 32 -> 3, per-node activations) on 8 TRN2 cores.

Data-parallel over the pixel axis. Each core processes P_CORE pixels laid out
feature-major as 4 pixel-groups on SBUF partitions:
  rhs partition (12*g + i) holds feature i of pixel-group g  (layer-1 input)
  hidden state partition layout per layer: 4 groups x 32 nodes, nodes sorted
  [gauss | sin | tanh-class] across groups.

All matmul data (x, weights, hidden state h) is fp16: full-rate PE matmuls,
half the DMA bytes, 10-bit mantissa (~5e-4 relative) which the 2e-2 harness
gate easily absorbs.  PSUM accumulation stays fp32.

The tanh-class (tanh/sigmoid/identity) is handled by ONE Tanh pass over all
128 partitions with per-partition scale/bias operands plus host-side
algebraic folds into the next layer's weights:
  sigmoid(z) = 0.5*tanh(z/2) + 0.5          (stored tanh(z/2); affine folded)
  identity(z) = tanh(eps*z)/eps             (stored tanh(eps*z); 1/eps folded)

Sin and gauss rows are only a fraction of the partitions but a sub-range
activation op costs the same as a full-height one (cost ~ free-dim length).
So the main Tanh pass writes those rows as the identity-eps encoding
tanh(eps*(u+b)) ~= eps*(u+b) (eps = 2^-9).  Each hidden level h lives in ONE
persistent ring tile [128, RING*CHUNK], so a whole pack-group of chunks is a
contiguous column slice: the class rows of pf consecutive chunks are moved
into a densely packed tile [4*n*pf, CHUNK] with a SINGLE gather DMA (the
partition/free reshape falls out of DMA flatten-order pairing), the per-class
op chains run once per packed tile (amortized pf-fold), and one scatter DMA
writes the results back over the eps-junk rows of all pf chunks:
  gauss(z) = exp(-(z+b)^2/2):  DVE squares the encoding (y = enc^2, fp32);
    t = Tanh((0.25/eps^2)*y) = tanh(((u+b)/2)^2);  gauss = 2/(1+t) - 1 via
    DVE add + reciprocal_approx_fast + one affine tensor_scalar (also the
    fp32->fp16 convert).  Square runs on DVE, not ScalarE: ScalarE is the
    bottleneck engine (3 main passes + sin + gauss tanh + out tanh).
  sin(z+b):  ADD_RANGE_WRAP wraps the encoding into [-eps*pi, eps*pi] (the
    wrap is linear so it works in eps-space; one period suffices since
    |z+b| < 3*pi), then Sin decodes with scale 1/eps.
Pack factors are divisors of RING so a group never wraps the ring.  Junk rows
above the packed region flow through every op harmlessly.  The output stores
write the full [128, 512] quadrant-packed tile per chunk-pair (junk rows
included) so each is ONE descriptor; the host unpacks.  DMA issue is spread
across sequencers (gauss gathers/scatters + output stores on SP's DGE; x
loads and sin gathers/scatters on GpSimd's; none on Activation) so no
sequencer's DIRECT2D issue cost (~0.7us each) starves ScalarE.
"""

import os
import sys

import numpy as np

_REPO = "/root/.axon_site/_ro/trn_rl_repo"
if _REPO not in sys.path and not os.path.isdir("/opt/trn_rl_repo"):
    sys.path.insert(0, _REPO)

import concourse.bacc as bacc
import concourse.bass as bass  # noqa: F401
import concourse.tile as tile
from concourse import mybir
from concourse.bass_utils import run_bass_kernel_spmd

# Pin the activation-function table to the single set containing every
# function this kernel uses ({Tanh, Sin}).  Without this, bacc's greedy
# per-instruction set selection can alternate between sets and emit an
# ACT_TABLE_LOAD (~2.7us) per chunk.
_orig_get_tables = bacc.get_activation_tables


def _pinned_tables(arch):
    t = _orig_get_tables(arch)
    if "silu_and_others" in t:
        # act_func_set_id is the POSITION in act_info.json's set list, so
        # keep every entry (order intact) and just empty the others.
        return {name: (funcs if name == "silu_and_others" else set())
                for name, funcs in t.items()}
    return t


bacc.get_activation_tables = _pinned_tables

F32 = mybir.dt.float32
F16 = mybir.dt.float16

P_TOTAL = 1024 * 1024
N_IN, H, N_OUT = 12, 32, 3
N_CORES = 8
P_CORE = P_TOTAL // N_CORES  # 131072
G = 4                        # pixel groups packed on partitions
PG = P_CORE // G             # 32768 pixels per group per core
CHUNK = 1024                 # pixels per group per chunk (2 PSUM banks)
MM_N = 512                   # matmul moving free dim (one PSUM bank)
RING = 12                    # h ring depth (chunks) per hidden level
ID_EPS = np.float32(2.0 ** -9)      # identity-via-tanh input scale
TWO_PI = float(2.0 * np.pi)
PI = float(np.pi)


def _pack_factor(n):
    """Chunks packed per class tile for a class with n nodes (4n rows).
    Must divide RING so groups never wrap the h ring."""
    if n == 0:
        return 0
    cap = 128 // (4 * n)
    for pf in (6, 4, 3, 2, 1):
        if pf <= cap:
            return pf
    return 1


# class codes: 0 = gauss, 1 = sin, 2 = tanh-class (tanh/sigmoid/identity)
def _cls_of_act(a):
    return {4: 0, 3: 1}.get(int(a), 2)


def _sorted_layout(act):
    """Order the H nodes by [gauss | sin | rest]; return (perm, n_gauss, n_sin).
    perm[j] = original node index placed at sorted slot j."""
    cls = np.array([_cls_of_act(a) for a in act])
    perm = np.argsort(cls, kind="stable")
    return perm, int((cls == 0).sum()), int((cls == 1).sum())


class _Plan:
    """Host-side folded weights + per-layer layouts. All float64 math."""

    def __init__(self, bias_in, W1, b1, act1, W2, b2, act2, W3, b3, act3,
                 Wout, bout):
        layers = [(W1, b1, act1), (W2, b2, act2), (W3, b3, act3)]
        self.perms, self.ngauss, self.nsin = [], [], []
        self.lhsT = []          # device stationary matrices (np.float32)
        self.cols = []          # per-layer dict of [128] operand columns
        # incoming per-node output transform: h_true = alpha*stored + beta
        in_alpha = np.ones(N_IN, dtype=np.float64)
        in_beta = np.asarray(bias_in, dtype=np.float64)  # h0 = x + bias_in
        in_dim = N_IN
        in_layout = None  # for L1 the input layout is the fixed feature order

        for li, (W, b, act) in enumerate(layers):
            W = np.asarray(W, dtype=np.float64)
            b = np.asarray(b, dtype=np.float64)
            act = np.asarray(act)
            perm, ng, ns = _sorted_layout(act)
            self.perms.append(perm)
            self.ngauss.append(ng)
            self.nsin.append(ns)

            # effective weights / bias absorbing incoming transforms
            W_eff = W * in_alpha[:, None]                  # [in_dim, H]
            b_eff = b + in_beta @ W                        # [H]

            # device stationary: block diagonal over groups with node sort
            K = G * in_dim
            lt = np.zeros((K, 128), dtype=np.float64)
            for g in range(G):
                for j in range(H):
                    node = perm[j]
                    m = self._row(li, g, j)
                    if li == 0:
                        rows = np.arange(in_dim) + in_dim * g
                        lt[rows, m] = W_eff[:, node]
                    else:
                        for k_in in range(in_dim):
                            kpart = in_layout[g][k_in]
                            lt[kpart, m] = W_eff[k_in, node]
            self.lhsT.append(lt.astype(np.float32))

            # operand columns.  Main tanh pass: per-partition scale/bias.
            tanh_scale = np.zeros(128, dtype=np.float64)
            tanh_bias = np.zeros(128, dtype=np.float64)
            out_alpha = np.ones(H, dtype=np.float64)
            out_beta = np.zeros(H, dtype=np.float64)
            for j in range(H):
                node = perm[j]
                a = int(act[node])
                be = b_eff[node]
                for g in range(G):
                    m = self._row(li, g, j)
                    if a == 1:        # tanh
                        tanh_scale[m] = 1.0
                        tanh_bias[m] = be
                    elif a == 2:      # sigmoid -> tanh(u/2)
                        tanh_scale[m] = 0.5
                        tanh_bias[m] = 0.5 * be
                    else:
                        # identity nodes AND the sin/gauss rows: the main
                        # tanh pass writes the identity-eps encoding
                        # tanh(eps*(u+b)) ~= eps*(u+b), which for sin/gauss
                        # is the value the packed chains gather from h
                        # (DMA cannot read PSUM).
                        tanh_scale[m] = float(ID_EPS)
                        tanh_bias[m] = float(ID_EPS) * be
                if a == 1:
                    out_alpha[node], out_beta[node] = 1.0, 0.0
                elif a == 2:
                    out_alpha[node], out_beta[node] = 0.5, 0.5
                elif a == 0:
                    out_alpha[node], out_beta[node] = 1.0 / float(ID_EPS), 0.0
                else:                 # sin / gauss: stored value is exact
                    out_alpha[node], out_beta[node] = 1.0, 0.0
            self.cols.append({
                "tanh_scale": tanh_scale, "tanh_bias": tanh_bias,
            })

            # next layer's incoming transform, in SORTED node order per device
            # partition -> but folds are per node; store per-node arrays and
            # the partition layout for the next lhsT build.
            in_alpha = out_alpha
            in_beta = out_beta
            in_dim = H
            # partition index of (g, sorted-slot j) for this layer's output
            in_layout = [[self._row(li, g, j) for j in range(H)]
                         for g in range(G)]
            # reorder alpha/beta to sorted-slot order for the next W_eff
            in_alpha = out_alpha[perm]
            in_beta = out_beta[perm]
            # next layer's W rows must be permuted accordingly
            if li < 2:
                layers[li + 1] = (np.asarray(layers[li + 1][0])[perm, :],
                                  layers[li + 1][1], layers[li + 1][2])
            else:
                self._wout_perm = perm

        # output layer
        Wo = np.asarray(Wout, dtype=np.float64)[self._wout_perm, :]
        bo = np.asarray(bout, dtype=np.float64)
        Wo_eff = Wo * in_alpha[:, None]
        bo_eff = bo + in_beta @ Wo
        lt = np.zeros((128, 32), dtype=np.float64)
        for g in range(G):
            for j in range(H):
                kpart = in_layout[g][j]
                for o in range(N_OUT):
                    lt[kpart, 3 * g + o] = Wo_eff[j, o]
        self.lhsT_out = lt.astype(np.float32)
        out_bias = np.zeros(128, dtype=np.float64)
        for q in range(4):
            for g in range(G):
                for o in range(N_OUT):
                    out_bias[32 * q + 3 * g + o] = bo_eff[o]
        self.out_bias = out_bias

        # pack all operand columns into one [128, 32] block
        colblk = np.zeros((128, 32), dtype=np.float64)
        for li in range(3):
            c = self.cols[li]
            colblk[:, 8 * li + 0] = c["tanh_scale"]
            colblk[:, 8 * li + 1] = c["tanh_bias"]
        colblk[:, 24] = self.out_bias
        self.colblk = colblk.astype(np.float32)

    @staticmethod
    def _row(li, g, j):
        """Device partition of sorted-slot j, group g (layer output layout).
        Rows are class-sorted ACROSS groups: slot j occupies partitions
        4*j + g."""
        return 4 * j + g


def _build_program(ngauss, nsin, p_core=P_CORE, chunk=CHUNK,
                   use_fp32r=True):
    """Build the bass module. Program structure depends only on the per-layer
    (n_gauss, n_sin) counts, not on weight values."""
    pg = p_core // G
    nchunk = pg // chunk
    nhalf = chunk // MM_N
    assert chunk % MM_N == 0 and pg % chunk == 0

    pfg = [_pack_factor(n) for n in ngauss]   # gauss pack factor per layer
    pfs = [_pack_factor(n) for n in nsin]     # sin pack factor per layer
    # emission skew between layers: covers each layer's largest pack-group
    # latency (a chunk's h completes only when its packed group completes;
    # skew is a priority hint, dataflow is dependency-enforced).  +1 for the
    # deferred chain phase B, +2 slack.
    s1 = max(pfg[0], pfs[0]) + 4
    s2 = s1 + max(pfg[1], pfs[1]) + 4
    s3 = s2 + max(pfg[2], pfs[2]) + 4
    skew = [0, s1, s2, s3]
    total_skew = skew[3] + 2

    nc = bacc.Bacc("TRN2", target_bir_lowering=False, debug=False,
                   num_devices=N_CORES)
    xT = nc.dram_tensor("xT", [G * N_IN, pg], F16, kind="ExternalInput").ap()
    wst = nc.dram_tensor("wst", [128, 416], F16, kind="ExternalInput").ap()
    cst = nc.dram_tensor("cst", [128, 64], F32, kind="ExternalInput").ap()
    yT = nc.dram_tensor("yT", [128, pg // 4], F32, kind="ExternalOutput").ap()

    with tile.TileContext(nc) as tc:
        cpool = tc.alloc_tile_pool(name="consts", bufs=1)
        wst_t = cpool.tile([128, 416], F16, tag="wst")
        cc_t = cpool.tile([128, 32], F32, tag="cc")
        nc.sync.dma_start(out=wst_t[:, 0:128], in_=wst[:, 0:128])
        nc.sync.dma_start(out=cc_t[:], in_=cst[:, 0:32])
        nc.sync.dma_start(out=wst_t[:, 128:416], in_=wst[:, 128:416])
        w1_t = wst_t[:, 0:128]
        w2_t = wst_t[:, 128:256]
        w3_t = wst_t[:, 256:384]
        wo_t = wst_t[:, 384:416]
        col_t = cc_t[:, 0:32]

        ring = min(RING, nchunk)
        # persistent h ring tiles, one per hidden level (subtile-dep tracked)
        h1_t = cpool.tile([128, ring * chunk], F16, tag="h1")
        h2_t = cpool.tile([128, ring * chunk], F16, tag="h2")
        h3_t = cpool.tile([128, ring * chunk], F16, tag="h3")
        h_ring = [None, h1_t, h2_t, h3_t]

        # one SBUF work pool + one PSUM pool (per-tag bufs); fewer pools =
        # fewer release-barrier ceremonies in the teardown
        wpool = tc.alloc_tile_pool(name="work", bufs=2)
        xpool = gpool = spool = scpool = rpool = opool = wpool
        ppool = tc.alloc_tile_pool(name="psum", bufs=3, space="PSUM")
        oppool = ppool

        w_tiles = [w1_t, w2_t, w3_t]
        x_live = {}     # chunk -> x tile
        pso_live = {}   # chunk-pair -> psum_o tile
        def _new_cst():
            return {"pend": [], "subs": [], "row": 0, "npf": 0, "done": 0,
                    "tile": None}

        gst = {li: _new_cst() for li in range(3)}
        sst = {li: _new_cst() for li in range(3)}

        def hsl(c, n=1):
            """Column slice of n consecutive chunks starting at c (no wrap:
            pack factors divide the ring depth)."""
            r = c % ring
            assert r + n <= ring
            return slice(r * chunk, (r + n) * chunk)

        def emit_load(c):
            x_t = xpool.tile([G * N_IN, chunk], F16, tag="x", bufs=4)
            nc.gpsimd.dma_start(
                out=x_t[:], in_=xT[:, c * chunk:(c + 1) * chunk])
            x_live[c] = x_t

        def emit_main(c, li):
            """Main matmuls + full-height tanh pass."""
            if li == 0:
                h_prev = x_live.pop(c)
            else:
                h_prev = h_ring[li][:, hsl(c)]
            kdim = G * N_IN if li == 0 else 128
            ps = ppool.tile([128, chunk], F32, tag="pre")
            wt = w_tiles[li]
            for hh in range(nhalf):
                sl = slice(hh * MM_N, (hh + 1) * MM_N)
                nc.tensor.matmul(
                    ps[:, sl],
                    wt[0:kdim, :],
                    h_prev[0:kdim, sl],
                    start=True, stop=True,
                )
            cb = 8 * li
            # tanh-class pass over all 128 rows (junk eps-encode on the
            # gauss/sin rows, overwritten by the packed-chain scatters)
            nc.scalar.activation(
                h_ring[li + 1][:, hsl(c)], ps[:],
                mybir.ActivationFunctionType.Tanh,
                bias=col_t[:, cb + 1:cb + 2],
                scale=col_t[:, cb + 0:cb + 1],
            )
            # incremental sub-group gathers into the packed class tiles:
            # each group of npf chunks is gathered in two sub-DMAs (rows
            # [off : off + 4n*L] <- [4n, L*C], the reshape falls out of the
            # DMA flatten-order pairing), so the packed data is ready right
            # after the group's last main pass and every AP is a contiguous
            # row range (soundly dependency-tracked).
            ng, ns = ngauss[li], nsin[li]
            if ng > 0:
                self_gather(gst[li], li, c, pfg[li], nc.sync, gpool,
                            f"gz{li}", 0, 4 * ng)
            if ns > 0:
                self_gather(sst[li], li, c, pfs[li], nc.gpsimd, spool,
                            f"sz{li}", 4 * ng, 4 * ns)

        def self_gather(st, li, c, pf, eng, pool, tag, rbase, rows):
            """Append chunk c to the class group; gather a sub when half the
            group (or the remainder) has accumulated."""
            if st["tile"] is None:
                st["tile"] = pool.tile([128, chunk], F16, tag=tag, name=tag)
                st["npf"] = min(pf, nchunk - c)
                st["row"] = 0
                st["done"] = 0
                st["subs"] = []
                st["pend"] = []
            st["pend"].append(c)
            # sub lengths: ceil(npf/2) then the rest
            first = (st["npf"] + 1) // 2
            want = first if st["done"] == 0 else st["npf"] - first
            if len(st["pend"]) == want:
                L = want
                c_start = st["pend"][0]
                off = st["row"]
                eng.dma_start(
                    out=st["tile"][off:off + rows * L, :],
                    in_=h_ring[li + 1][rbase:rbase + rows, hsl(c_start, L)])
                st["subs"].append((off, c_start, L))
                st["row"] = off + rows * L
                st["done"] += L
                st["pend"] = []

        def scatter_subs(eng, res, li, rbase, rows, subs):
            for off, c_start, L in subs:
                eng.dma_start(
                    out=h_ring[li + 1][rbase:rbase + rows, hsl(c_start, L)],
                    in_=res[off:off + rows * L, :])

        def gauss_head(li):
            """Phase A: DVE square of the packed encodings (gathers already
            landed incrementally).  Returns phase-B state."""
            st = gst[li]
            R = st["row"]
            gz = st["tile"]
            subs = st["subs"]
            st["tile"] = None
            # y = enc^2 on DVE (fp32; the (0.25/eps^2) decode folds into the
            # Tanh scale) - keeps Square off the bottleneck ScalarE
            ysq = scpool.tile([128, chunk], F32, tag="gy", bufs=6)
            nc.vector.tensor_tensor(ysq[0:R, :], gz[0:R, :], gz[0:R, :],
                                    mybir.AluOpType.mult)
            return (li, ysq, subs, R)

        def gauss_act(st):
            """Phase B1: t = tanh(((u+b)/2)^2) on ScalarE."""
            li, ysq, subs, R = st
            t_t = scpool.tile([128, chunk], F32, tag="gt", bufs=3)
            nc.scalar.activation(
                t_t[0:R, :], ysq[0:R, :], mybir.ActivationFunctionType.Tanh,
                scale=float(0.25 / (ID_EPS * ID_EPS)),
            )
            return (li, t_t, subs, R)

        def gauss_tail(st):
            """Phase B2: den = 1 + t ; r = 1/den ; out = 2r - 1 =
            exp(-(z+b)^2/2); scatter the subs back."""
            li, t_t, subs, R = st
            # den = 1 + t, in place (DVE element-wise streaming)
            nc.vector.tensor_scalar(
                t_t[0:R, :], t_t[0:R, :], 1.0, None, mybir.AluOpType.add)
            rin_t = scpool.tile([128, chunk], F32, tag="gr", bufs=3)
            nc.vector.reciprocal_approx_fast(rin_t[0:R, :], t_t[0:R, :])
            g_r = rpool.tile([128, chunk], F16, tag="go", bufs=3)
            nc.vector.tensor_scalar(
                g_r[0:R, :], rin_t[0:R, :], 2.0, -1.0,
                mybir.AluOpType.mult, mybir.AluOpType.add)
            scatter_subs(nc.sync, g_r, li, 0, 4 * ngauss[li], subs)

        def sin_head(li):
            """Phase A: DVE range-wrap of the packed encodings in eps-space."""
            st = sst[li]
            R = st["row"]
            sz = st["tile"]
            subs = st["subs"]
            st["tile"] = None
            m_t = scpool.tile([128, chunk], F32, tag="sm", bufs=6)
            nc.vector.add_range_wrap(
                m_t[0:R, :], sz[0:R, :],
                0.0, float(ID_EPS) * PI, float(ID_EPS) * TWO_PI)
            return (li, m_t, subs, R)

        def sin_act(st):
            """Phase B: Sin decodes with scale 1/eps; scatter the subs."""
            li, m_t, subs, R = st
            s_r = rpool.tile([128, chunk], F16, tag="so", bufs=3)
            nc.scalar.activation(
                s_r[0:R, :], m_t[0:R, :], mybir.ActivationFunctionType.Sin,
                scale=float(1.0 / ID_EPS))
            scatter_subs(nc.gpsimd, s_r, li, 4 * ngauss[li], 4 * nsin[li],
                         subs)

        def collect_flushes(c, li, heads):
            """After emit_main(c, li): start phase A for completed groups."""
            if ngauss[li] > 0 and gst[li]["tile"] is not None \
                    and gst[li]["done"] == gst[li]["npf"]:
                heads.append(("g", gauss_head(li)))
            if nsin[li] > 0 and sst[li]["tile"] is not None \
                    and sst[li]["done"] == sst[li]["npf"]:
                heads.append(("s", sin_head(li)))

        def emit_out(c):
            # output layer: quadrant-packed [12,512] matmuls into a quad
            # psum tile [128, 1024] (2 banks); one Tanh pass + one store per
            # 4 chunks (flat yT layout identical to the per-pair variant)
            h_prev = h_ring[3][:, hsl(c)]
            q0 = 2 * (c % 2)
            off = ((c % 4) // 2) * MM_N
            if c % 4 == 0:
                pso_live[c // 4] = oppool.tile([128, 2 * MM_N], F32,
                                               tag="preo", name="pso",
                                               bufs=1)
            pso = pso_live[c // 4]
            for hh in range(nhalf):
                q = q0 + hh
                nc.tensor.matmul(
                    pso[32 * q:32 * q + 32, off:off + MM_N],
                    wo_t,
                    h_prev[:, hh * MM_N:(hh + 1) * MM_N],
                    start=True, stop=True,
                    tile_position=(0, 32 * q),
                )
            if c % 4 == 3 or c == nchunk - 1:
                pso_live.pop(c // 4)
                ncols = off + MM_N
                osb = opool.tile([128, 2 * MM_N], F32, tag="osb")
                nc.scalar.activation(
                    osb[:, 0:ncols], pso[:, 0:ncols],
                    mybir.ActivationFunctionType.Tanh,
                    bias=col_t[:, 24:25],
                )
                k = c // 4
                nc.sync.dma_start(
                    out=yT[:, k * 2 * MM_N:k * 2 * MM_N + ncols],
                    in_=osb[:, 0:ncols])

        # Software-pipelined emission with per-layer skew covering the packed
        # group latency.  Step order [L2, L3, L1, out] keeps the out-layer
        # matmuls (which wait on L3 chain scatters) from head-of-line
        # blocking L1's matmuls in the PE FIFO, and defers each chain's
        # ScalarE op (phase B) one full step behind its DVE head (phase A)
        # so it is ready when the ACT queue reaches it.
        assert nchunk % 2 == 0
        emit_load(0)
        pending = []   # phase-A states from the previous step
        for t in range(nchunk + total_skew):
            heads = []
            if skew[1] <= t and t - skew[1] < nchunk:
                emit_main(t - skew[1], 1)
                collect_flushes(t - skew[1], 1, heads)
            if skew[2] <= t and t - skew[2] < nchunk:
                emit_main(t - skew[2], 2)
                collect_flushes(t - skew[2], 2, heads)
            if t + 1 < nchunk:
                emit_load(t + 1)
            if t < nchunk:
                emit_main(t, 0)
                collect_flushes(t, 0, heads)
            if skew[3] <= t and t - skew[3] < nchunk:
                emit_out(t - skew[3])
            # phase B for last step's groups: ACT parts first, tails after
            bstates = []
            for kind, st in pending:
                bstates.append((kind, gauss_act(st) if kind == "g" else st))
            for kind, st in bstates:
                if kind == "g":
                    gauss_tail(st)
                else:
                    sin_act(st)
            pending = heads

        for p in (ppool, wpool, cpool):
            p.release()

    nc.compile()
    return nc


_PROGRAM_CACHE = {}


def _get_program(ngauss, nsin, p_core=P_CORE, chunk=CHUNK, use_fp32r=True):
    key = (tuple(ngauss), tuple(nsin), p_core, chunk, use_fp32r)
    if key not in _PROGRAM_CACHE:
        _PROGRAM_CACHE[key] = _build_program(ngauss, nsin, p_core, chunk,
                                             use_fp32r=use_fp32r)
    return _PROGRAM_CACHE[key]


def make_in_maps(inputs, plan, p_core=P_CORE, n_cores=N_CORES):
    """Shard + transpose the pixel data; replicate constants."""
    x = np.ascontiguousarray(np.asarray(inputs["inputs"], dtype=np.float32))
    pg = p_core // G
    in_maps = []
    for core in range(n_cores):
        xc = x[core * p_core:(core + 1) * p_core]          # [p_core, 12]
        xg = xc.reshape(G, pg, N_IN)                        # [G, pg, 12]
        xT = np.ascontiguousarray(xg.transpose(0, 2, 1)
                                  .reshape(G * N_IN, pg)
                                  .astype(np.float16))      # [48, pg]
        wst = np.zeros((128, 416), dtype=np.float16)
        wst[0:G * N_IN, 0:128] = plan.lhsT[0]
        wst[:, 128:256] = plan.lhsT[1]
        wst[:, 256:384] = plan.lhsT[2]
        wst[:, 384:416] = plan.lhsT_out
        cst = np.zeros((128, 64), dtype=np.float32)
        cst[:, 0:32] = plan.colblk
        in_maps.append({"xT": xT, "wst": wst, "cst": cst})
    return in_maps


def assemble_output(results, p_core=P_CORE, n_cores=N_CORES):
    pg = p_core // G
    nk = pg // (2 * CHUNK) if pg >= 2 * CHUNK else 1
    out = np.empty((p_core * n_cores, N_OUT), dtype=np.float32)
    for core in range(n_cores):
        yT = results[core]["yT"]                     # [128, pg/4]
        # quadrant layout: yT[32q + 3g + o, 512k + j] is (g, o) of slot
        # s = 2048k + 512q + j
        arr = yT.reshape(4, 32, nk, MM_N)            # [q, row, k, j]
        yc = (arr[:, 0:12]                           # [q, 3g+o, k, j]
              .transpose(1, 2, 0, 3)                 # [3g+o, k, q, j]
              .reshape(G, N_OUT, pg)                 # [g, o, slot]
              .transpose(0, 2, 1))                   # [g, slot, o]
        out[core * p_core:(core + 1) * p_core] = yc.reshape(p_core, N_OUT)
    return out


def make_plan(inputs):
    return _Plan(
        inputs["bias_in"], inputs["W1"], inputs["b1"], inputs["act1"],
        inputs["W2"], inputs["b2"], inputs["act2"],
        inputs["W3"], inputs["b3"], inputs["act3"],
        inputs["Wout"], inputs["bout"])


def run(inputs, trace=False, use_fp32r=True, **spmd_kwargs):
    plan = make_plan(inputs)
    nc = _get_program(plan.ngauss, plan.nsin, use_fp32r=use_fp32r)
    in_maps = make_in_maps(inputs, plan)
    res = run_bass_kernel_spmd(nc, in_maps, list(range(N_CORES)),
                               trace=trace, **spmd_kwargs)
    return assemble_output(res.results), res


def kernel(**inputs) -> np.ndarray:
    out, _ = run(inputs, trace=False)
    return out


# revision 36
# speedup vs baseline: 1.0661x; 1.0077x over previous
"""CPPN MLP (12 -> 32 -> 32 -> 32 -> 3, per-node activations) on 8 TRN2 cores.

Data-parallel over the pixel axis. Each core processes P_CORE pixels laid out
feature-major as 4 pixel-groups on SBUF partitions:
  rhs partition (12*g + i) holds feature i of pixel-group g  (layer-1 input)
  hidden state partition layout per layer: 4 groups x 32 nodes, nodes sorted
  [gauss | sin | tanh-class] across groups.

All matmul data (x, weights, hidden state h) is fp16: full-rate PE matmuls,
half the DMA bytes, 10-bit mantissa (~5e-4 relative) which the 2e-2 harness
gate easily absorbs.  PSUM accumulation stays fp32.

The tanh-class (tanh/sigmoid/identity) is handled by ONE Tanh pass over all
128 partitions with per-partition scale/bias operands plus host-side
algebraic folds into the next layer's weights:
  sigmoid(z) = 0.5*tanh(z/2) + 0.5          (stored tanh(z/2); affine folded)
  identity(z) = tanh(eps*z)/eps             (stored tanh(eps*z); 1/eps folded)

Sin and gauss rows are only a fraction of the partitions but a sub-range
activation op costs the same as a full-height one (cost ~ free-dim length).
So the main Tanh pass writes those rows as the identity-eps encoding
tanh(eps*(u+b)) ~= eps*(u+b) (eps = 2^-9).  Each hidden level h lives in ONE
persistent ring tile [128, RING*CHUNK], so a pack-group of pf consecutive
chunks is a contiguous column slice: its class rows are moved into a densely
packed tile [4*n*pf, CHUNK] with two sub-group gather DMAs ([4n, L*C] ->
[4n*L, C]; the partition/free reshape falls out of DMA flatten-order
pairing, and the incremental gathers mean the packed tile is complete the
moment the group's last main pass is), the per-class op chains run once per
packed tile (amortized pf-fold), and sub-group scatter DMAs write the
results back over the eps-junk rows of all pf chunks.  Each chain's ScalarE
op is deferred one emission step behind its gather + DVE head so the strict
in-order ACT queue never stalls on it; per-step emission order [L2, L3, L1,
out] keeps the out-layer matmuls (gated by L3 chain scatters) from
head-of-line blocking L1's matmuls in the PE FIFO:
  gauss(z) = exp(-(z+b)^2/2):  DVE squares the encoding (y = enc^2, fp32);
    t = Tanh((0.25/eps^2)*y) = tanh(((u+b)/2)^2);  gauss = 2/(1+t) - 1 via
    DVE add + reciprocal_approx_fast + one affine tensor_scalar (also the
    fp32->fp16 convert).  Square runs on DVE, not ScalarE: ScalarE is the
    bottleneck engine (3 main passes + sin + gauss tanh + out tanh).
  sin(z+b):  ADD_RANGE_WRAP wraps the encoding into [-eps*pi, eps*pi] (the
    wrap is linear so it works in eps-space; one period suffices since
    |z+b| < 3*pi), then Sin decodes with scale 1/eps.
Pack factors are divisors of RING so a group never wraps the ring.  Junk rows
above the packed region flow through every op harmlessly.  The output layer
quad-packs four chunks' quadrant matmuls into one [128, 1024] psum tile, so
there is ONE full-height Tanh pass and ONE store per 4 chunks (junk rows
included); the host unpacks.  DMA issue is spread across sequencers (gauss
gathers/scatters + output stores on SP's DGE; x loads and sin
gathers/scatters on GpSimd's; none on Activation) so no sequencer's
DIRECT2D issue cost (~0.7us each) starves ScalarE.
"""

import os
import sys

import numpy as np

_REPO = "/root/.axon_site/_ro/trn_rl_repo"
if _REPO not in sys.path and not os.path.isdir("/opt/trn_rl_repo"):
    sys.path.insert(0, _REPO)

import concourse.bacc as bacc
import concourse.bass as bass  # noqa: F401
import concourse.tile as tile
from concourse import mybir
from concourse.bass_utils import run_bass_kernel_spmd

# Pin the activation-function table to the single set containing every
# function this kernel uses ({Tanh, Sin}).  Without this, bacc's greedy
# per-instruction set selection can alternate between sets and emit an
# ACT_TABLE_LOAD (~2.7us) per chunk.
_orig_get_tables = bacc.get_activation_tables


def _pinned_tables(arch):
    t = _orig_get_tables(arch)
    if "silu_and_others" in t:
        # act_func_set_id is the POSITION in act_info.json's set list, so
        # keep every entry (order intact) and just empty the others.
        return {name: (funcs if name == "silu_and_others" else set())
                for name, funcs in t.items()}
    return t


bacc.get_activation_tables = _pinned_tables

F32 = mybir.dt.float32
F16 = mybir.dt.float16

P_TOTAL = 1024 * 1024
N_IN, H, N_OUT = 12, 32, 3
N_CORES = 8
P_CORE = P_TOTAL // N_CORES  # 131072
G = 4                        # pixel groups packed on partitions
PG = P_CORE // G             # 32768 pixels per group per core
CHUNK = 1024                 # pixels per group per chunk (2 PSUM banks)
MM_N = 512                   # matmul moving free dim (one PSUM bank)
RING = 12                    # h ring depth (chunks) per hidden level
ID_EPS = np.float32(2.0 ** -9)      # identity-via-tanh input scale
TWO_PI = float(2.0 * np.pi)
PI = float(np.pi)


def _pack_factor(n):
    """Chunks packed per class tile for a class with n nodes (4n rows).
    Must divide RING so groups never wrap the h ring."""
    if n == 0:
        return 0
    cap = 128 // (4 * n)
    for pf in (6, 4, 3, 2, 1):
        if pf <= cap:
            return pf
    return 1


# class codes: 0 = gauss, 1 = sin, 2 = tanh-class (tanh/sigmoid/identity)
def _cls_of_act(a):
    return {4: 0, 3: 1}.get(int(a), 2)


def _sorted_layout(act):
    """Order the H nodes by [gauss | sin | rest]; return (perm, n_gauss, n_sin).
    perm[j] = original node index placed at sorted slot j."""
    cls = np.array([_cls_of_act(a) for a in act])
    perm = np.argsort(cls, kind="stable")
    return perm, int((cls == 0).sum()), int((cls == 1).sum())


class _Plan:
    """Host-side folded weights + per-layer layouts. All float64 math."""

    def __init__(self, bias_in, W1, b1, act1, W2, b2, act2, W3, b3, act3,
                 Wout, bout):
        layers = [(W1, b1, act1), (W2, b2, act2), (W3, b3, act3)]
        self.perms, self.ngauss, self.nsin = [], [], []
        self.lhsT = []          # device stationary matrices (np.float32)
        self.cols = []          # per-layer dict of [128] operand columns
        # incoming per-node output transform: h_true = alpha*stored + beta
        in_alpha = np.ones(N_IN, dtype=np.float64)
        in_beta = np.asarray(bias_in, dtype=np.float64)  # h0 = x + bias_in
        in_dim = N_IN
        in_layout = None  # for L1 the input layout is the fixed feature order

        for li, (W, b, act) in enumerate(layers):
            W = np.asarray(W, dtype=np.float64)
            b = np.asarray(b, dtype=np.float64)
            act = np.asarray(act)
            perm, ng, ns = _sorted_layout(act)
            self.perms.append(perm)
            self.ngauss.append(ng)
            self.nsin.append(ns)

            # effective weights / bias absorbing incoming transforms
            W_eff = W * in_alpha[:, None]                  # [in_dim, H]
            b_eff = b + in_beta @ W                        # [H]

            # device stationary: block diagonal over groups with node sort
            K = G * in_dim
            lt = np.zeros((K, 128), dtype=np.float64)
            for g in range(G):
                for j in range(H):
                    node = perm[j]
                    m = self._row(li, g, j)
                    if li == 0:
                        rows = np.arange(in_dim) + in_dim * g
                        lt[rows, m] = W_eff[:, node]
                    else:
                        for k_in in range(in_dim):
                            kpart = in_layout[g][k_in]
                            lt[kpart, m] = W_eff[k_in, node]
            self.lhsT.append(lt.astype(np.float32))

            # operand columns.  Main tanh pass: per-partition scale/bias.
            tanh_scale = np.zeros(128, dtype=np.float64)
            tanh_bias = np.zeros(128, dtype=np.float64)
            out_alpha = np.ones(H, dtype=np.float64)
            out_beta = np.zeros(H, dtype=np.float64)
            for j in range(H):
                node = perm[j]
                a = int(act[node])
                be = b_eff[node]
                for g in range(G):
                    m = self._row(li, g, j)
                    if a == 1:        # tanh
                        tanh_scale[m] = 1.0
                        tanh_bias[m] = be
                    elif a == 2:      # sigmoid -> tanh(u/2)
                        tanh_scale[m] = 0.5
                        tanh_bias[m] = 0.5 * be
                    else:
                        # identity nodes AND the sin/gauss rows: the main
                        # tanh pass writes the identity-eps encoding
                        # tanh(eps*(u+b)) ~= eps*(u+b), which for sin/gauss
                        # is the value the packed chains gather from h
                        # (DMA cannot read PSUM).
                        tanh_scale[m] = float(ID_EPS)
                        tanh_bias[m] = float(ID_EPS) * be
                if a == 1:
                    out_alpha[node], out_beta[node] = 1.0, 0.0
                elif a == 2:
                    out_alpha[node], out_beta[node] = 0.5, 0.5
                elif a == 0:
                    out_alpha[node], out_beta[node] = 1.0 / float(ID_EPS), 0.0
                else:                 # sin / gauss: stored value is exact
                    out_alpha[node], out_beta[node] = 1.0, 0.0
            self.cols.append({
                "tanh_scale": tanh_scale, "tanh_bias": tanh_bias,
            })

            # next layer's incoming transform, in SORTED node order per device
            # partition -> but folds are per node; store per-node arrays and
            # the partition layout for the next lhsT build.
            in_alpha = out_alpha
            in_beta = out_beta
            in_dim = H
            # partition index of (g, sorted-slot j) for this layer's output
            in_layout = [[self._row(li, g, j) for j in range(H)]
                         for g in range(G)]
            # reorder alpha/beta to sorted-slot order for the next W_eff
            in_alpha = out_alpha[perm]
            in_beta = out_beta[perm]
            # next layer's W rows must be permuted accordingly
            if li < 2:
                layers[li + 1] = (np.asarray(layers[li + 1][0])[perm, :],
                                  layers[li + 1][1], layers[li + 1][2])
            else:
                self._wout_perm = perm

        # output layer
        Wo = np.asarray(Wout, dtype=np.float64)[self._wout_perm, :]
        bo = np.asarray(bout, dtype=np.float64)
        Wo_eff = Wo * in_alpha[:, None]
        bo_eff = bo + in_beta @ Wo
        lt = np.zeros((128, 32), dtype=np.float64)
        for g in range(G):
            for j in range(H):
                kpart = in_layout[g][j]
                for o in range(N_OUT):
                    lt[kpart, 3 * g + o] = Wo_eff[j, o]
        self.lhsT_out = lt.astype(np.float32)
        out_bias = np.zeros(128, dtype=np.float64)
        for q in range(4):
            for g in range(G):
                for o in range(N_OUT):
                    out_bias[32 * q + 3 * g + o] = bo_eff[o]
        self.out_bias = out_bias

        # pack all operand columns into one [128, 32] block
        colblk = np.zeros((128, 32), dtype=np.float64)
        for li in range(3):
            c = self.cols[li]
            colblk[:, 8 * li + 0] = c["tanh_scale"]
            colblk[:, 8 * li + 1] = c["tanh_bias"]
        colblk[:, 24] = self.out_bias
        self.colblk = colblk.astype(np.float32)

    @staticmethod
    def _row(li, g, j):
        """Device partition of sorted-slot j, group g (layer output layout).
        Rows are class-sorted ACROSS groups: slot j occupies partitions
        4*j + g."""
        return 4 * j + g


def _build_program(ngauss, nsin, p_core=P_CORE, chunk=CHUNK,
                   use_fp32r=True):
    """Build the bass module. Program structure depends only on the per-layer
    (n_gauss, n_sin) counts, not on weight values."""
    pg = p_core // G
    nchunk = pg // chunk
    nhalf = chunk // MM_N
    assert chunk % MM_N == 0 and pg % chunk == 0

    pfg = [_pack_factor(n) for n in ngauss]   # gauss pack factor per layer
    pfs = [_pack_factor(n) for n in nsin]     # sin pack factor per layer
    # emission skew between layers: covers each layer's largest pack-group
    # latency (a chunk's h completes only when its packed group completes;
    # skew is a priority hint, dataflow is dependency-enforced).  +1 for the
    # deferred chain phase B, +2 slack.
    s1 = max(pfg[0], pfs[0]) + 4
    s2 = s1 + max(pfg[1], pfs[1]) + 4
    s3 = s2 + max(pfg[2], pfs[2]) + 4
    skew = [0, s1, s2, s3]
    total_skew = skew[3] + 2

    nc = bacc.Bacc("TRN2", target_bir_lowering=False, debug=False,
                   num_devices=N_CORES)
    xT = nc.dram_tensor("xT", [G * N_IN, pg], F16, kind="ExternalInput").ap()
    wst = nc.dram_tensor("wst", [128, 416], F16, kind="ExternalInput").ap()
    cst = nc.dram_tensor("cst", [128, 64], F32, kind="ExternalInput").ap()
    yT = nc.dram_tensor("yT", [128, pg // 4], F32, kind="ExternalOutput").ap()

    with tile.TileContext(nc) as tc:
        cpool = tc.alloc_tile_pool(name="consts", bufs=1)
        wst_t = cpool.tile([128, 416], F16, tag="wst")
        cc_t = cpool.tile([128, 32], F32, tag="cc")
        nc.sync.dma_start(out=wst_t[:, 0:128], in_=wst[:, 0:128])
        nc.sync.dma_start(out=cc_t[:], in_=cst[:, 0:32])
        nc.sync.dma_start(out=wst_t[:, 128:416], in_=wst[:, 128:416])
        w1_t = wst_t[:, 0:128]
        w2_t = wst_t[:, 128:256]
        w3_t = wst_t[:, 256:384]
        wo_t = wst_t[:, 384:416]
        col_t = cc_t[:, 0:32]

        ring = min(RING, nchunk)
        # persistent h ring tiles, one per hidden level (subtile-dep tracked)
        h1_t = cpool.tile([128, ring * chunk], F16, tag="h1")
        h2_t = cpool.tile([128, ring * chunk], F16, tag="h2")
        h3_t = cpool.tile([128, ring * chunk], F16, tag="h3")
        h_ring = [None, h1_t, h2_t, h3_t]

        # one SBUF work pool + one PSUM pool (per-tag bufs); fewer pools =
        # fewer release-barrier ceremonies in the teardown
        wpool = tc.alloc_tile_pool(name="work", bufs=2)
        xpool = gpool = spool = scpool = rpool = opool = wpool
        ppool = tc.alloc_tile_pool(name="psum", bufs=3, space="PSUM")
        oppool = ppool

        w_tiles = [w1_t, w2_t, w3_t]
        x_live = {}     # chunk -> x tile
        pso_live = {}   # chunk-pair -> psum_o tile
        def _new_cst():
            return {"pend": [], "subs": [], "row": 0, "npf": 0, "done": 0,
                    "tile": None}

        gst = {li: _new_cst() for li in range(3)}
        sst = {li: _new_cst() for li in range(3)}

        def hsl(c, n=1):
            """Column slice of n consecutive chunks starting at c (no wrap:
            pack factors divide the ring depth)."""
            r = c % ring
            assert r + n <= ring
            return slice(r * chunk, (r + n) * chunk)

        def emit_load(c):
            x_t = xpool.tile([G * N_IN, chunk], F16, tag="x", bufs=4)
            nc.gpsimd.dma_start(
                out=x_t[:], in_=xT[:, c * chunk:(c + 1) * chunk])
            x_live[c] = x_t

        def emit_main(c, li):
            """Main matmuls + full-height tanh pass."""
            if li == 0:
                h_prev = x_live.pop(c)
            else:
                h_prev = h_ring[li][:, hsl(c)]
            kdim = G * N_IN if li == 0 else 128
            ps = ppool.tile([128, chunk], F32, tag="pre")
            wt = w_tiles[li]
            for hh in range(nhalf):
                sl = slice(hh * MM_N, (hh + 1) * MM_N)
                nc.tensor.matmul(
                    ps[:, sl],
                    wt[0:kdim, :],
                    h_prev[0:kdim, sl],
                    start=True, stop=True,
                )
            cb = 8 * li
            # tanh-class pass over all 128 rows (junk eps-encode on the
            # gauss/sin rows, overwritten by the packed-chain scatters)
            nc.scalar.activation(
                h_ring[li + 1][:, hsl(c)], ps[:],
                mybir.ActivationFunctionType.Tanh,
                bias=col_t[:, cb + 1:cb + 2],
                scale=col_t[:, cb + 0:cb + 1],
            )
            # incremental sub-group gathers into the packed class tiles:
            # each group of npf chunks is gathered in two sub-DMAs (rows
            # [off : off + 4n*L] <- [4n, L*C], the reshape falls out of the
            # DMA flatten-order pairing), so the packed data is ready right
            # after the group's last main pass and every AP is a contiguous
            # row range (soundly dependency-tracked).
            ng, ns = ngauss[li], nsin[li]
            if ng > 0:
                self_gather(gst[li], li, c, pfg[li], nc.sync, gpool,
                            f"gz{li}", 0, 4 * ng)
            if ns > 0:
                self_gather(sst[li], li, c, pfs[li], nc.gpsimd, spool,
                            f"sz{li}", 4 * ng, 4 * ns)

        def self_gather(st, li, c, pf, eng, pool, tag, rbase, rows):
            """Append chunk c to the class group; gather a sub when half the
            group (or the remainder) has accumulated."""
            if st["tile"] is None:
                st["tile"] = pool.tile([128, chunk], F16, tag=tag, name=tag)
                st["npf"] = min(pf, nchunk - c)
                st["row"] = 0
                st["done"] = 0
                st["subs"] = []
                st["pend"] = []
            st["pend"].append(c)
            # sub lengths: ceil(npf/2) then the rest
            first = (st["npf"] + 1) // 2
            want = first if st["done"] == 0 else st["npf"] - first
            if len(st["pend"]) == want:
                L = want
                c_start = st["pend"][0]
                off = st["row"]
                eng.dma_start(
                    out=st["tile"][off:off + rows * L, :],
                    in_=h_ring[li + 1][rbase:rbase + rows, hsl(c_start, L)])
                st["subs"].append((off, c_start, L))
                st["row"] = off + rows * L
                st["done"] += L
                st["pend"] = []

        def scatter_subs(eng, res, li, rbase, rows, subs):
            for off, c_start, L in subs:
                eng.dma_start(
                    out=h_ring[li + 1][rbase:rbase + rows, hsl(c_start, L)],
                    in_=res[off:off + rows * L, :])

        def gauss_head(li):
            """Phase A: DVE square of the packed encodings (gathers already
            landed incrementally).  Returns phase-B state."""
            st = gst[li]
            R = st["row"]
            gz = st["tile"]
            subs = st["subs"]
            st["tile"] = None
            # y = enc^2 on DVE (fp32; the (0.25/eps^2) decode folds into the
            # Tanh scale) - keeps Square off the bottleneck ScalarE
            ysq = scpool.tile([128, chunk], F32, tag="gy", bufs=6)
            nc.vector.tensor_tensor(ysq[0:R, :], gz[0:R, :], gz[0:R, :],
                                    mybir.AluOpType.mult)
            return (li, ysq, subs, R)

        def gauss_act(st):
            """Phase B1: t = tanh(((u+b)/2)^2) on ScalarE."""
            li, ysq, subs, R = st
            t_t = scpool.tile([128, chunk], F32, tag="gt", bufs=3)
            nc.scalar.activation(
                t_t[0:R, :], ysq[0:R, :], mybir.ActivationFunctionType.Tanh,
                scale=float(0.25 / (ID_EPS * ID_EPS)),
            )
            return (li, t_t, subs, R)

        def gauss_tail(st):
            """Phase B2: den = 1 + t ; r = 1/den ; out = 2r - 1 =
            exp(-(z+b)^2/2); scatter the subs back."""
            li, t_t, subs, R = st
            # den = 1 + t, in place (DVE element-wise streaming)
            nc.vector.tensor_scalar(
                t_t[0:R, :], t_t[0:R, :], 1.0, None, mybir.AluOpType.add)
            rin_t = scpool.tile([128, chunk], F32, tag="gr", bufs=3)
            nc.vector.reciprocal_approx_fast(rin_t[0:R, :], t_t[0:R, :])
            g_r = rpool.tile([128, chunk], F16, tag="go", bufs=3)
            nc.vector.tensor_scalar(
                g_r[0:R, :], rin_t[0:R, :], 2.0, -1.0,
                mybir.AluOpType.mult, mybir.AluOpType.add)
            scatter_subs(nc.sync, g_r, li, 0, 4 * ngauss[li], subs)

        def sin_head(li):
            """Phase A: DVE range-wrap of the packed encodings in eps-space."""
            st = sst[li]
            R = st["row"]
            sz = st["tile"]
            subs = st["subs"]
            st["tile"] = None
            m_t = scpool.tile([128, chunk], F32, tag="sm", bufs=6)
            nc.vector.add_range_wrap(
                m_t[0:R, :], sz[0:R, :],
                0.0, float(ID_EPS) * PI, float(ID_EPS) * TWO_PI)
            return (li, m_t, subs, R)

        def sin_act(st):
            """Phase B: Sin decodes with scale 1/eps; scatter the subs."""
            li, m_t, subs, R = st
            s_r = rpool.tile([128, chunk], F16, tag="so", bufs=3)
            nc.scalar.activation(
                s_r[0:R, :], m_t[0:R, :], mybir.ActivationFunctionType.Sin,
                scale=float(1.0 / ID_EPS))
            scatter_subs(nc.gpsimd, s_r, li, 4 * ngauss[li], 4 * nsin[li],
                         subs)

        def collect_flushes(c, li, heads):
            """After emit_main(c, li): start phase A for completed groups."""
            if ngauss[li] > 0 and gst[li]["tile"] is not None \
                    and gst[li]["done"] == gst[li]["npf"]:
                heads.append(("g", gauss_head(li)))
            if nsin[li] > 0 and sst[li]["tile"] is not None \
                    and sst[li]["done"] == sst[li]["npf"]:
                heads.append(("s", sin_head(li)))

        def emit_out(c):
            # output layer: quadrant-packed [12,512] matmuls into a quad
            # psum tile [128, 1024] (2 banks); one Tanh pass + one store per
            # 4 chunks (flat yT layout identical to the per-pair variant)
            h_prev = h_ring[3][:, hsl(c)]
            q0 = 2 * (c % 2)
            off = ((c % 4) // 2) * MM_N
            if c % 4 == 0:
                pso_live[c // 4] = oppool.tile([128, 2 * MM_N], F32,
                                               tag="preo", name="pso",
                                               bufs=1)
            pso = pso_live[c // 4]
            for hh in range(nhalf):
                q = q0 + hh
                nc.tensor.matmul(
                    pso[32 * q:32 * q + 32, off:off + MM_N],
                    wo_t,
                    h_prev[:, hh * MM_N:(hh + 1) * MM_N],
                    start=True, stop=True,
                    tile_position=(0, 32 * q),
                )
            if c % 4 == 3 or c == nchunk - 1:
                pso_live.pop(c // 4)
                ncols = off + MM_N
                osb = opool.tile([128, 2 * MM_N], F32, tag="osb")
                nc.scalar.activation(
                    osb[:, 0:ncols], pso[:, 0:ncols],
                    mybir.ActivationFunctionType.Tanh,
                    bias=col_t[:, 24:25],
                )
                k = c // 4
                nc.sync.dma_start(
                    out=yT[:, k * 2 * MM_N:k * 2 * MM_N + ncols],
                    in_=osb[:, 0:ncols])

        # Software-pipelined emission with per-layer skew covering the packed
        # group latency.  Step order [L2, L3, L1, out] keeps the out-layer
        # matmuls (which wait on L3 chain scatters) from head-of-line
        # blocking L1's matmuls in the PE FIFO, and defers each chain's
        # ScalarE op (phase B) one full step behind its DVE head (phase A)
        # so it is ready when the ACT queue reaches it.
        assert nchunk % 2 == 0
        emit_load(0)
        pending = []   # phase-A states from the previous step
        for t in range(nchunk + total_skew):
            heads = []
            if skew[1] <= t and t - skew[1] < nchunk:
                emit_main(t - skew[1], 1)
                collect_flushes(t - skew[1], 1, heads)
            if skew[2] <= t and t - skew[2] < nchunk:
                emit_main(t - skew[2], 2)
                collect_flushes(t - skew[2], 2, heads)
            if t + 1 < nchunk:
                emit_load(t + 1)
            if t < nchunk:
                emit_main(t, 0)
                collect_flushes(t, 0, heads)
            if skew[3] <= t and t - skew[3] < nchunk:
                emit_out(t - skew[3])
            # phase B for last step's groups: ACT parts first, tails after
            bstates = []
            for kind, st in pending:
                bstates.append((kind, gauss_act(st) if kind == "g" else st))
            for kind, st in bstates:
                if kind == "g":
                    gauss_tail(st)
                else:
                    sin_act(st)
            pending = heads

        for p in (ppool, wpool, cpool):
            p.release()

    nc.compile()
    return nc


_PROGRAM_CACHE = {}


def _get_program(ngauss, nsin, p_core=P_CORE, chunk=CHUNK, use_fp32r=True):
    key = (tuple(ngauss), tuple(nsin), p_core, chunk, use_fp32r)
    if key not in _PROGRAM_CACHE:
        _PROGRAM_CACHE[key] = _build_program(ngauss, nsin, p_core, chunk,
                                             use_fp32r=use_fp32r)
    return _PROGRAM_CACHE[key]


def make_in_maps(inputs, plan, p_core=P_CORE, n_cores=N_CORES):
    """Shard + transpose the pixel data; replicate constants."""
    x = np.ascontiguousarray(np.asarray(inputs["inputs"], dtype=np.float32))
    pg = p_core // G
    in_maps = []
    for core in range(n_cores):
        xc = x[core * p_core:(core + 1) * p_core]          # [p_core, 12]
        xg = xc.reshape(G, pg, N_IN)                        # [G, pg, 12]
        xT = np.ascontiguousarray(xg.transpose(0, 2, 1)
                                  .reshape(G * N_IN, pg)
                                  .astype(np.float16))      # [48, pg]
        wst = np.zeros((128, 416), dtype=np.float16)
        wst[0:G * N_IN, 0:128] = plan.lhsT[0]
        wst[:, 128:256] = plan.lhsT[1]
        wst[:, 256:384] = plan.lhsT[2]
        wst[:, 384:416] = plan.lhsT_out
        cst = np.zeros((128, 64), dtype=np.float32)
        cst[:, 0:32] = plan.colblk
        in_maps.append({"xT": xT, "wst": wst, "cst": cst})
    return in_maps


def assemble_output(results, p_core=P_CORE, n_cores=N_CORES):
    pg = p_core // G
    nk = pg // (2 * CHUNK) if pg >= 2 * CHUNK else 1
    out = np.empty((p_core * n_cores, N_OUT), dtype=np.float32)
    for core in range(n_cores):
        yT = results[core]["yT"]                     # [128, pg/4]
        # quadrant layout: yT[32q + 3g + o, 512k + j] is (g, o) of slot
        # s = 2048k + 512q + j
        arr = yT.reshape(4, 32, nk, MM_N)            # [q, row, k, j]
        yc = (arr[:, 0:12]                           # [q, 3g+o, k, j]
              .transpose(1, 2, 0, 3)                 # [3g+o, k, q, j]
              .reshape(G, N_OUT, pg)                 # [g, o, slot]
              .transpose(0, 2, 1))                   # [g, slot, o]
        out[core * p_core:(core + 1) * p_core] = yc.reshape(p_core, N_OUT)
    return out


def make_plan(inputs):
    return _Plan(
        inputs["bias_in"], inputs["W1"], inputs["b1"], inputs["act1"],
        inputs["W2"], inputs["b2"], inputs["act2"],
        inputs["W3"], inputs["b3"], inputs["act3"],
        inputs["Wout"], inputs["bout"])


def run(inputs, trace=False, use_fp32r=True, **spmd_kwargs):
    plan = make_plan(inputs)
    nc = _get_program(plan.ngauss, plan.nsin, use_fp32r=use_fp32r)
    in_maps = make_in_maps(inputs, plan)
    res = run_bass_kernel_spmd(nc, in_maps, list(range(N_CORES)),
                               trace=trace, **spmd_kwargs)
    return assemble_output(res.results), res


def kernel(**inputs) -> np.ndarray:
    out, _ = run(inputs, trace=False)
    return out


# revision 37
# speedup vs baseline: 1.1227x; 1.0531x over previous
"""CPPN MLP (12 -> 32 -> 32 -> 32 -> 3, per-node activations) on 8 TRN2 cores.

Data-parallel over the pixel axis. Each core processes P_CORE pixels laid out
feature-major as 4 pixel-groups on SBUF partitions:
  rhs partition (12*g + i) holds feature i of pixel-group g  (layer-1 input)
  hidden state partition layout per layer: 4 groups x 32 nodes, nodes sorted
  [gauss | sin | tanh-class] across groups.

All matmul data (x, weights, hidden state h) is fp16: full-rate PE matmuls,
half the DMA bytes, 10-bit mantissa (~5e-4 relative) which the 2e-2 harness
gate easily absorbs.  PSUM accumulation stays fp32.

The tanh-class (tanh/sigmoid/identity) is handled by ONE Tanh pass over all
128 partitions with per-partition scale/bias operands plus host-side
algebraic folds into the next layer's weights:
  sigmoid(z) = 0.5*tanh(z/2) + 0.5          (stored tanh(z/2); affine folded)
  identity(z) = tanh(eps*z)/eps             (stored tanh(eps*z); 1/eps folded)

Sin and gauss rows are only a fraction of the partitions but a sub-range
activation op costs the same as a full-height one (cost ~ free-dim length).
So the main Tanh pass writes those rows as the identity-eps encoding
tanh(eps*(u+b)) ~= eps*(u+b) (eps = 2^-9).  Each hidden level h lives in ONE
persistent ring tile [128, RING*CHUNK], so a pack-group of pf consecutive
chunks is a contiguous column slice: its class rows are moved into a densely
packed tile [4*n*pf, CHUNK] with two sub-group gather DMAs ([4n, L*C] ->
[4n*L, C]; the partition/free reshape falls out of DMA flatten-order
pairing, and the incremental gathers mean the packed tile is complete the
moment the group's last main pass is), the per-class op chains run once per
packed tile (amortized pf-fold), and sub-group scatter DMAs write the
results back over the eps-junk rows of all pf chunks.  Each chain's ScalarE
op is deferred one emission step behind its gather + DVE head so the strict
in-order ACT queue never stalls on it; per-step emission order [L2, L3, L1,
out] keeps the out-layer matmuls (gated by L3 chain scatters) from
head-of-line blocking L1's matmuls in the PE FIFO:
  gauss(z) = exp(-(z+b)^2/2):  DVE squares the encoding (y = enc^2, fp32);
    t = Tanh((0.25/eps^2)*y) = tanh(((u+b)/2)^2);  gauss = 2/(1+t) - 1 via
    DVE add + reciprocal_approx_fast + one affine tensor_scalar (also the
    fp32->fp16 convert).  Square runs on DVE, not ScalarE: ScalarE is the
    bottleneck engine (3 main passes + sin + gauss tanh + out tanh).
  sin(z+b):  ADD_RANGE_WRAP wraps the encoding into [-eps*pi, eps*pi] (the
    wrap is linear so it works in eps-space; one period suffices since
    |z+b| < 3*pi), then Sin decodes with scale 1/eps.
Pack factors are divisors of RING so a group never wraps the ring.  Junk rows
above the packed region flow through every op harmlessly.  The output layer
quad-packs four chunks' quadrant matmuls into one [128, 1024] psum tile, so
there is ONE full-height Tanh pass and ONE store per 4 chunks (junk rows
included); the host unpacks.  DMA issue is spread across sequencers (gauss
gathers/scatters + output stores on SP's DGE; x loads and sin
gathers/scatters on GpSimd's; none on Activation) so no sequencer's
DIRECT2D issue cost (~0.7us each) starves ScalarE.
"""

import os
import sys

import numpy as np

_REPO = "/root/.axon_site/_ro/trn_rl_repo"
if _REPO not in sys.path and not os.path.isdir("/opt/trn_rl_repo"):
    sys.path.insert(0, _REPO)

import concourse.bacc as bacc
import concourse.bass as bass  # noqa: F401
import concourse.tile as tile
from concourse import mybir
from concourse.bass_utils import run_bass_kernel_spmd

# Pin the activation-function table to the single set containing every
# function this kernel uses ({Tanh, Sin}).  Without this, bacc's greedy
# per-instruction set selection can alternate between sets and emit an
# ACT_TABLE_LOAD (~2.7us) per chunk.
_orig_get_tables = bacc.get_activation_tables


def _pinned_tables(arch):
    t = _orig_get_tables(arch)
    if "silu_and_others" in t:
        # act_func_set_id is the POSITION in act_info.json's set list, so
        # keep every entry (order intact) and just empty the others.
        return {name: (funcs if name == "silu_and_others" else set())
                for name, funcs in t.items()}
    return t


bacc.get_activation_tables = _pinned_tables

F32 = mybir.dt.float32
F16 = mybir.dt.float16

P_TOTAL = 1024 * 1024
N_IN, H, N_OUT = 12, 32, 3
N_CORES = 8
P_CORE = P_TOTAL // N_CORES  # 131072
G = 4                        # pixel groups packed on partitions
PG = P_CORE // G             # 32768 pixels per group per core
CHUNK = 1024                 # pixels per group per chunk (2 PSUM banks)
MM_N = 512                   # matmul moving free dim (one PSUM bank)
RING = 12                    # h ring depth (chunks) per hidden level
ID_EPS = np.float32(2.0 ** -9)      # identity-via-tanh input scale
TWO_PI = float(2.0 * np.pi)
PI = float(np.pi)


def _pack_factor(n):
    """Chunks packed per class tile for a class with n nodes (4n rows).
    Must divide RING so groups never wrap the h ring."""
    if n == 0:
        return 0
    cap = 128 // (4 * n)
    for pf in (6, 4, 3, 2, 1):
        if pf <= cap:
            return pf
    return 1


# class codes: 0 = gauss, 1 = sin, 2 = tanh-class (tanh/sigmoid/identity)
def _cls_of_act(a):
    return {4: 0, 3: 1}.get(int(a), 2)


def _sorted_layout(act):
    """Order the H nodes by [gauss | sin | rest]; return (perm, n_gauss, n_sin).
    perm[j] = original node index placed at sorted slot j."""
    cls = np.array([_cls_of_act(a) for a in act])
    perm = np.argsort(cls, kind="stable")
    return perm, int((cls == 0).sum()), int((cls == 1).sum())


class _Plan:
    """Host-side folded weights + per-layer layouts. All float64 math."""

    def __init__(self, bias_in, W1, b1, act1, W2, b2, act2, W3, b3, act3,
                 Wout, bout):
        layers = [(W1, b1, act1), (W2, b2, act2), (W3, b3, act3)]
        self.perms, self.ngauss, self.nsin = [], [], []
        self.lhsT = []          # device stationary matrices (np.float32)
        self.cols = []          # per-layer dict of [128] operand columns
        # incoming per-node output transform: h_true = alpha*stored + beta
        in_alpha = np.ones(N_IN, dtype=np.float64)
        in_beta = np.asarray(bias_in, dtype=np.float64)  # h0 = x + bias_in
        in_dim = N_IN
        in_layout = None  # for L1 the input layout is the fixed feature order

        for li, (W, b, act) in enumerate(layers):
            W = np.asarray(W, dtype=np.float64)
            b = np.asarray(b, dtype=np.float64)
            act = np.asarray(act)
            perm, ng, ns = _sorted_layout(act)
            self.perms.append(perm)
            self.ngauss.append(ng)
            self.nsin.append(ns)

            # effective weights / bias absorbing incoming transforms
            W_eff = W * in_alpha[:, None]                  # [in_dim, H]
            b_eff = b + in_beta @ W                        # [H]

            # device stationary: block diagonal over groups with node sort
            K = G * in_dim
            lt = np.zeros((K, 128), dtype=np.float64)
            for g in range(G):
                for j in range(H):
                    node = perm[j]
                    m = self._row(li, g, j)
                    if li == 0:
                        rows = np.arange(in_dim) + in_dim * g
                        lt[rows, m] = W_eff[:, node]
                    else:
                        for k_in in range(in_dim):
                            kpart = in_layout[g][k_in]
                            lt[kpart, m] = W_eff[k_in, node]
            self.lhsT.append(lt.astype(np.float32))

            # operand columns.  Main tanh pass: per-partition scale/bias.
            tanh_scale = np.zeros(128, dtype=np.float64)
            tanh_bias = np.zeros(128, dtype=np.float64)
            out_alpha = np.ones(H, dtype=np.float64)
            out_beta = np.zeros(H, dtype=np.float64)
            for j in range(H):
                node = perm[j]
                a = int(act[node])
                be = b_eff[node]
                for g in range(G):
                    m = self._row(li, g, j)
                    if a == 1:        # tanh
                        tanh_scale[m] = 1.0
                        tanh_bias[m] = be
                    elif a == 2:      # sigmoid -> tanh(u/2)
                        tanh_scale[m] = 0.5
                        tanh_bias[m] = 0.5 * be
                    else:
                        # identity nodes AND the sin/gauss rows: the main
                        # tanh pass writes the identity-eps encoding
                        # tanh(eps*(u+b)) ~= eps*(u+b), which for sin/gauss
                        # is the value the packed chains gather from h
                        # (DMA cannot read PSUM).
                        tanh_scale[m] = float(ID_EPS)
                        tanh_bias[m] = float(ID_EPS) * be
                if a == 1:
                    out_alpha[node], out_beta[node] = 1.0, 0.0
                elif a == 2:
                    out_alpha[node], out_beta[node] = 0.5, 0.5
                elif a == 0:
                    out_alpha[node], out_beta[node] = 1.0 / float(ID_EPS), 0.0
                else:                 # sin / gauss: stored value is exact
                    out_alpha[node], out_beta[node] = 1.0, 0.0
            self.cols.append({
                "tanh_scale": tanh_scale, "tanh_bias": tanh_bias,
            })

            # next layer's incoming transform, in SORTED node order per device
            # partition -> but folds are per node; store per-node arrays and
            # the partition layout for the next lhsT build.
            in_alpha = out_alpha
            in_beta = out_beta
            in_dim = H
            # partition index of (g, sorted-slot j) for this layer's output
            in_layout = [[self._row(li, g, j) for j in range(H)]
                         for g in range(G)]
            # reorder alpha/beta to sorted-slot order for the next W_eff
            in_alpha = out_alpha[perm]
            in_beta = out_beta[perm]
            # next layer's W rows must be permuted accordingly
            if li < 2:
                layers[li + 1] = (np.asarray(layers[li + 1][0])[perm, :],
                                  layers[li + 1][1], layers[li + 1][2])
            else:
                self._wout_perm = perm

        # output layer
        Wo = np.asarray(Wout, dtype=np.float64)[self._wout_perm, :]
        bo = np.asarray(bout, dtype=np.float64)
        Wo_eff = Wo * in_alpha[:, None]
        bo_eff = bo + in_beta @ Wo
        lt = np.zeros((128, 32), dtype=np.float64)
        for g in range(G):
            for j in range(H):
                kpart = in_layout[g][j]
                for o in range(N_OUT):
                    lt[kpart, 3 * g + o] = Wo_eff[j, o]
        self.lhsT_out = lt.astype(np.float32)
        out_bias = np.zeros(128, dtype=np.float64)
        for q in range(4):
            for g in range(G):
                for o in range(N_OUT):
                    out_bias[32 * q + 3 * g + o] = bo_eff[o]
        self.out_bias = out_bias

        # pack all operand columns into one [128, 32] block
        colblk = np.zeros((128, 32), dtype=np.float64)
        for li in range(3):
            c = self.cols[li]
            colblk[:, 8 * li + 0] = c["tanh_scale"]
            colblk[:, 8 * li + 1] = c["tanh_bias"]
        colblk[:, 24] = self.out_bias
        self.colblk = colblk.astype(np.float32)

    @staticmethod
    def _row(li, g, j):
        """Device partition of sorted-slot j, group g (layer output layout).
        Rows are class-sorted ACROSS groups: slot j occupies partitions
        4*j + g."""
        return 4 * j + g


def _build_program(ngauss, nsin, p_core=P_CORE, chunk=CHUNK,
                   use_fp32r=True):
    """Build the bass module. Program structure depends only on the per-layer
    (n_gauss, n_sin) counts, not on weight values."""
    pg = p_core // G
    nchunk = pg // chunk
    nhalf = chunk // MM_N
    assert chunk % MM_N == 0 and pg % chunk == 0

    pfg = [_pack_factor(n) for n in ngauss]   # gauss pack factor per layer
    pfs = [_pack_factor(n) for n in nsin]     # sin pack factor per layer
    # emission skew between layers: covers each layer's largest pack-group
    # latency (a chunk's h completes only when its packed group completes;
    # skew is a priority hint, dataflow is dependency-enforced).  +1 for the
    # deferred chain phase B, +2 slack.
    s1 = max(pfg[0], pfs[0]) + 6
    s2 = s1 + max(pfg[1], pfs[1]) + 4
    s3 = s2 + max(pfg[2], pfs[2]) + 4
    skew = [0, s1, s2, s3]
    total_skew = skew[3] + 2

    nc = bacc.Bacc("TRN2", target_bir_lowering=False, debug=False,
                   num_devices=N_CORES)
    xT = nc.dram_tensor("xT", [G * N_IN, pg], F16, kind="ExternalInput").ap()
    wst = nc.dram_tensor("wst", [128, 416], F16, kind="ExternalInput").ap()
    cst = nc.dram_tensor("cst", [128, 64], F32, kind="ExternalInput").ap()
    yT = nc.dram_tensor("yT", [128, pg // 4], F32, kind="ExternalOutput").ap()

    with tile.TileContext(nc) as tc:
        cpool = tc.alloc_tile_pool(name="consts", bufs=1)
        wst_t = cpool.tile([128, 416], F16, tag="wst")
        cc_t = cpool.tile([128, 32], F32, tag="cc")
        nc.sync.dma_start(out=wst_t[:, 0:128], in_=wst[:, 0:128])
        nc.sync.dma_start(out=cc_t[:], in_=cst[:, 0:32])
        nc.sync.dma_start(out=wst_t[:, 128:416], in_=wst[:, 128:416])
        w1_t = wst_t[:, 0:128]
        w2_t = wst_t[:, 128:256]
        w3_t = wst_t[:, 256:384]
        wo_t = wst_t[:, 384:416]
        col_t = cc_t[:, 0:32]

        ring = min(RING, nchunk)
        # persistent h ring tiles, one per hidden level (subtile-dep tracked)
        h1_t = cpool.tile([128, ring * chunk], F16, tag="h1")
        h2_t = cpool.tile([128, ring * chunk], F16, tag="h2")
        h3_t = cpool.tile([128, ring * chunk], F16, tag="h3")
        h_ring = [None, h1_t, h2_t, h3_t]

        # one SBUF work pool + one PSUM pool (per-tag bufs); fewer pools =
        # fewer release-barrier ceremonies in the teardown
        wpool = tc.alloc_tile_pool(name="work", bufs=2)
        xpool = gpool = spool = scpool = rpool = opool = wpool
        ppool = tc.alloc_tile_pool(name="psum", bufs=3, space="PSUM")
        oppool = ppool

        w_tiles = [w1_t, w2_t, w3_t]
        x_live = {}     # chunk -> x tile
        pso_live = {}   # chunk-pair -> psum_o tile
        def _new_cst():
            return {"pend": [], "subs": [], "row": 0, "npf": 0, "done": 0,
                    "tile": None}

        gst = {li: _new_cst() for li in range(3)}
        sst = {li: _new_cst() for li in range(3)}

        def hsl(c, n=1):
            """Column slice of n consecutive chunks starting at c (no wrap:
            pack factors divide the ring depth)."""
            r = c % ring
            assert r + n <= ring
            return slice(r * chunk, (r + n) * chunk)

        def emit_load(c):
            x_t = xpool.tile([G * N_IN, chunk], F16, tag="x", bufs=4)
            nc.gpsimd.dma_start(
                out=x_t[:], in_=xT[:, c * chunk:(c + 1) * chunk])
            x_live[c] = x_t

        def emit_main(c, li):
            """Main matmuls + full-height tanh pass."""
            if li == 0:
                h_prev = x_live.pop(c)
            else:
                h_prev = h_ring[li][:, hsl(c)]
            kdim = G * N_IN if li == 0 else 128
            ps = ppool.tile([128, chunk], F32, tag="pre")
            wt = w_tiles[li]
            for hh in range(nhalf):
                sl = slice(hh * MM_N, (hh + 1) * MM_N)
                nc.tensor.matmul(
                    ps[:, sl],
                    wt[0:kdim, :],
                    h_prev[0:kdim, sl],
                    start=True, stop=True,
                )
            cb = 8 * li
            # tanh-class pass over all 128 rows (junk eps-encode on the
            # gauss/sin rows, overwritten by the packed-chain scatters)
            nc.scalar.activation(
                h_ring[li + 1][:, hsl(c)], ps[:],
                mybir.ActivationFunctionType.Tanh,
                bias=col_t[:, cb + 1:cb + 2],
                scale=col_t[:, cb + 0:cb + 1],
            )
            # incremental sub-group gathers into the packed class tiles:
            # each group of npf chunks is gathered in two sub-DMAs (rows
            # [off : off + 4n*L] <- [4n, L*C], the reshape falls out of the
            # DMA flatten-order pairing), so the packed data is ready right
            # after the group's last main pass and every AP is a contiguous
            # row range (soundly dependency-tracked).
            ng, ns = ngauss[li], nsin[li]
            if ng > 0:
                self_gather(gst[li], li, c, pfg[li], nc.sync, gpool,
                            f"gz{li}", 0, 4 * ng)
            if ns > 0:
                self_gather(sst[li], li, c, pfs[li], nc.gpsimd, spool,
                            f"sz{li}", 4 * ng, 4 * ns)

        def self_gather(st, li, c, pf, eng, pool, tag, rbase, rows):
            """Append chunk c to the class group; gather a sub when half the
            group (or the remainder) has accumulated."""
            if st["tile"] is None:
                st["tile"] = pool.tile([128, chunk], F16, tag=tag, name=tag)
                st["npf"] = min(pf, nchunk - c)
                st["row"] = 0
                st["done"] = 0
                st["subs"] = []
                st["pend"] = []
            st["pend"].append(c)
            # sub lengths: ceil(npf/2) then the rest
            first = (st["npf"] + 1) // 2
            want = first if st["done"] == 0 else st["npf"] - first
            if len(st["pend"]) == want:
                L = want
                c_start = st["pend"][0]
                off = st["row"]
                eng.dma_start(
                    out=st["tile"][off:off + rows * L, :],
                    in_=h_ring[li + 1][rbase:rbase + rows, hsl(c_start, L)])
                st["subs"].append((off, c_start, L))
                st["row"] = off + rows * L
                st["done"] += L
                st["pend"] = []

        def scatter_subs(eng, res, li, rbase, rows, subs):
            for off, c_start, L in subs:
                eng.dma_start(
                    out=h_ring[li + 1][rbase:rbase + rows, hsl(c_start, L)],
                    in_=res[off:off + rows * L, :])

        def gauss_head(li):
            """Phase A: DVE square of the packed encodings (gathers already
            landed incrementally).  Returns phase-B state."""
            st = gst[li]
            R = st["row"]
            gz = st["tile"]
            subs = st["subs"]
            st["tile"] = None
            # y = enc^2 on DVE (fp32; the (0.25/eps^2) decode folds into the
            # Tanh scale) - keeps Square off the bottleneck ScalarE
            ysq = scpool.tile([128, chunk], F32, tag="gy", bufs=6)
            nc.vector.tensor_tensor(ysq[0:R, :], gz[0:R, :], gz[0:R, :],
                                    mybir.AluOpType.mult)
            return (li, ysq, subs, R)

        def gauss_act(st):
            """Phase B1: t = tanh(((u+b)/2)^2) on ScalarE."""
            li, ysq, subs, R = st
            t_t = scpool.tile([128, chunk], F32, tag="gt", bufs=3)
            nc.scalar.activation(
                t_t[0:R, :], ysq[0:R, :], mybir.ActivationFunctionType.Tanh,
                scale=float(0.25 / (ID_EPS * ID_EPS)),
            )
            return (li, t_t, subs, R)

        def gauss_tail(st):
            """Phase B2: den = 1 + t ; r = 1/den ; out = 2r - 1 =
            exp(-(z+b)^2/2); scatter the subs back."""
            li, t_t, subs, R = st
            # den = 1 + t, in place (DVE element-wise streaming)
            nc.vector.tensor_scalar(
                t_t[0:R, :], t_t[0:R, :], 1.0, None, mybir.AluOpType.add)
            rin_t = scpool.tile([128, chunk], F32, tag="gr", bufs=3)
            nc.vector.reciprocal_approx_fast(rin_t[0:R, :], t_t[0:R, :])
            g_r = rpool.tile([128, chunk], F16, tag="go", bufs=3)
            nc.vector.tensor_scalar(
                g_r[0:R, :], rin_t[0:R, :], 2.0, -1.0,
                mybir.AluOpType.mult, mybir.AluOpType.add)
            scatter_subs(nc.sync, g_r, li, 0, 4 * ngauss[li], subs)

        def sin_head(li):
            """Phase A: DVE range-wrap of the packed encodings in eps-space."""
            st = sst[li]
            R = st["row"]
            sz = st["tile"]
            subs = st["subs"]
            st["tile"] = None
            m_t = scpool.tile([128, chunk], F32, tag="sm", bufs=6)
            nc.vector.add_range_wrap(
                m_t[0:R, :], sz[0:R, :],
                0.0, float(ID_EPS) * PI, float(ID_EPS) * TWO_PI)
            return (li, m_t, subs, R)

        def sin_act(st):
            """Phase B: Sin decodes with scale 1/eps; scatter the subs."""
            li, m_t, subs, R = st
            s_r = rpool.tile([128, chunk], F16, tag="so", bufs=3)
            nc.scalar.activation(
                s_r[0:R, :], m_t[0:R, :], mybir.ActivationFunctionType.Sin,
                scale=float(1.0 / ID_EPS))
            scatter_subs(nc.gpsimd, s_r, li, 4 * ngauss[li], 4 * nsin[li],
                         subs)

        def collect_flushes(c, li, heads):
            """After emit_main(c, li): start phase A for completed groups."""
            if ngauss[li] > 0 and gst[li]["tile"] is not None \
                    and gst[li]["done"] == gst[li]["npf"]:
                heads.append(("g", gauss_head(li)))
            if nsin[li] > 0 and sst[li]["tile"] is not None \
                    and sst[li]["done"] == sst[li]["npf"]:
                heads.append(("s", sin_head(li)))

        def emit_out(c):
            # output layer: quadrant-packed [12,512] matmuls into a quad
            # psum tile [128, 1024] (2 banks); one Tanh pass + one store per
            # 4 chunks (flat yT layout identical to the per-pair variant)
            h_prev = h_ring[3][:, hsl(c)]
            q0 = 2 * (c % 2)
            off = ((c % 4) // 2) * MM_N
            if c % 4 == 0:
                pso_live[c // 4] = oppool.tile([128, 2 * MM_N], F32,
                                               tag="preo", name="pso",
                                               bufs=1)
            pso = pso_live[c // 4]
            for hh in range(nhalf):
                q = q0 + hh
                nc.tensor.matmul(
                    pso[32 * q:32 * q + 32, off:off + MM_N],
                    wo_t,
                    h_prev[:, hh * MM_N:(hh + 1) * MM_N],
                    start=True, stop=True,
                    tile_position=(0, 32 * q),
                )
            if c % 4 == 3 or c == nchunk - 1:
                pso_live.pop(c // 4)
                ncols = off + MM_N
                osb = opool.tile([128, 2 * MM_N], F32, tag="osb")
                nc.scalar.activation(
                    osb[:, 0:ncols], pso[:, 0:ncols],
                    mybir.ActivationFunctionType.Tanh,
                    bias=col_t[:, 24:25],
                )
                k = c // 4
                nc.sync.dma_start(
                    out=yT[:, k * 2 * MM_N:k * 2 * MM_N + ncols],
                    in_=osb[:, 0:ncols])

        # Software-pipelined emission with per-layer skew covering the packed
        # group latency.  Step order [L2, L3, L1, out] keeps the out-layer
        # matmuls (which wait on L3 chain scatters) from head-of-line
        # blocking L1's matmuls in the PE FIFO, and defers each chain's
        # ScalarE op (phase B) one full step behind its DVE head (phase A)
        # so it is ready when the ACT queue reaches it.
        assert nchunk % 2 == 0
        emit_load(0)
        pending = []   # phase-A states from the previous step
        for t in range(nchunk + total_skew):
            heads = []
            if skew[1] <= t and t - skew[1] < nchunk:
                emit_main(t - skew[1], 1)
                collect_flushes(t - skew[1], 1, heads)
            if skew[2] <= t and t - skew[2] < nchunk:
                emit_main(t - skew[2], 2)
                collect_flushes(t - skew[2], 2, heads)
            if t + 1 < nchunk:
                emit_load(t + 1)
            if t < nchunk:
                emit_main(t, 0)
                collect_flushes(t, 0, heads)
            if skew[3] <= t and t - skew[3] < nchunk:
                emit_out(t - skew[3])
            # phase B for last step's groups: ACT parts first, tails after
            bstates = []
            for kind, st in pending:
                bstates.append((kind, gauss_act(st) if kind == "g" else st))
            for kind, st in bstates:
                if kind == "g":
                    gauss_tail(st)
                else:
                    sin_act(st)
            pending = heads

        for p in (ppool, wpool, cpool):
            p.release()

    nc.compile()
    return nc


_PROGRAM_CACHE = {}


def _get_program(ngauss, nsin, p_core=P_CORE, chunk=CHUNK, use_fp32r=True):
    key = (tuple(ngauss), tuple(nsin), p_core, chunk, use_fp32r)
    if key not in _PROGRAM_CACHE:
        _PROGRAM_CACHE[key] = _build_program(ngauss, nsin, p_core, chunk,
                                             use_fp32r=use_fp32r)
    return _PROGRAM_CACHE[key]


def make_in_maps(inputs, plan, p_core=P_CORE, n_cores=N_CORES):
    """Shard + transpose the pixel data; replicate constants."""
    x = np.ascontiguousarray(np.asarray(inputs["inputs"], dtype=np.float32))
    pg = p_core // G
    in_maps = []
    for core in range(n_cores):
        xc = x[core * p_core:(core + 1) * p_core]          # [p_core, 12]
        xg = xc.reshape(G, pg, N_IN)                        # [G, pg, 12]
        xT = np.ascontiguousarray(xg.transpose(0, 2, 1)
                                  .reshape(G * N_IN, pg)
                                  .astype(np.float16))      # [48, pg]
        wst = np.zeros((128, 416), dtype=np.float16)
        wst[0:G * N_IN, 0:128] = plan.lhsT[0]
        wst[:, 128:256] = plan.lhsT[1]
        wst[:, 256:384] = plan.lhsT[2]
        wst[:, 384:416] = plan.lhsT_out
        cst = np.zeros((128, 64), dtype=np.float32)
        cst[:, 0:32] = plan.colblk
        in_maps.append({"xT": xT, "wst": wst, "cst": cst})
    return in_maps


def assemble_output(results, p_core=P_CORE, n_cores=N_CORES):
    pg = p_core // G
    nk = pg // (2 * CHUNK) if pg >= 2 * CHUNK else 1
    out = np.empty((p_core * n_cores, N_OUT), dtype=np.float32)
    for core in range(n_cores):
        yT = results[core]["yT"]                     # [128, pg/4]
        # quadrant layout: yT[32q + 3g + o, 512k + j] is (g, o) of slot
        # s = 2048k + 512q + j
        arr = yT.reshape(4, 32, nk, MM_N)            # [q, row, k, j]
        yc = (arr[:, 0:12]                           # [q, 3g+o, k, j]
              .transpose(1, 2, 0, 3)                 # [3g+o, k, q, j]
              .reshape(G, N_OUT, pg)                 # [g, o, slot]
              .transpose(0, 2, 1))                   # [g, slot, o]
        out[core * p_core:(core + 1) * p_core] = yc.reshape(p_core, N_OUT)
    return out


def make_plan(inputs):
    return _Plan(
        inputs["bias_in"], inputs["W1"], inputs["b1"], inputs["act1"],
        inputs["W2"], inputs["b2"], inputs["act2"],
        inputs["W3"], inputs["b3"], inputs["act3"],
        inputs["Wout"], inputs["bout"])


def run(inputs, trace=False, use_fp32r=True, **spmd_kwargs):
    plan = make_plan(inputs)
    nc = _get_program(plan.ngauss, plan.nsin, use_fp32r=use_fp32r)
    in_maps = make_in_maps(inputs, plan)
    res = run_bass_kernel_spmd(nc, in_maps, list(range(N_CORES)),
                               trace=trace, **spmd_kwargs)
    return assemble_output(res.results), res


def kernel(**inputs) -> np.ndarray:
    out, _ = run(inputs, trace=False)
    return out
